# revision 12
# baseline (speedup 1.0000x reference)
"""GQA attention (bs=2, seq=2048, dim=2048, 16 q-heads / 8 kv-heads, hd=128)
on 8 Trainium2 NeuronCores.

Sharding: 2-way data parallel (batch) x 4-way tensor parallel (heads, kv
groups intact).  Core c handles batch c//4 and q-heads [4*(c%4), 4*(c%4)+4)
(kv-heads [2*(c%4), 2*(c%4)+2)).  Each core computes a partial output
projection (row-split wo); the all-reduce over the 4 TP ranks is done on the
host while gathering (bf16 partials summed in f32).

Device kernel (per core):
  - all inputs bf16 (weights, x^T) -> FWL-eligible stationaries, half DMA.
  - host supplies X^T (so `dim` lands on partitions for every projection)
    and rotate-half permuted wq/wk, so RoPE is 4 DVE ops per tile.
  - scores are computed transposed (P^T[k, q]) which makes PV and the
    output projection transpose-free.
  - causal masking is additive: a [128,128] -1e9 strictly-lower matrix is
    accumulated into the scores PSUM bank by a tiny N=128 matmul
    (identity stationary) before the score matmul, so exp() produces
    exact zeros and the DVE mask multiply disappears from the
    exp->PV chain.
  - softmax row-sums: P^T chunks are accumulated into a [128, QB] fp16
    SBUF tile by DVE adds; one all-ones [128,1] matmul per head-block
    reduces over partitions; 1/z = exp(-ln(z)) on the scalar engine
    (Ln and Exp share one ACT table set), avoiding any DMA round-trip;
    a [1,128] ones matmul broadcasts 1/z back to 128 partitions for the
    DVE normalization multiply.

Perf notes (vs the first working version, 312.4us -> target ~270us):
  - 96 warmup matmuls (>3.4us busy) so the PE HAM clock-gate opens at
    ~3.4us instead of 50us; previously the whole DMA-fed ramp ran at
    1.2GHz.
  - startup DMAs spread over 4 engine queues (scalar/vector for wq,
    sync/gpsimd for x block 0) and block-0 Q accumulates all 4 heads
    per d-chunk, so the PE consumes each 2x128KB chunk-pair (863ns) at
    the pace DMA delivers it.
  - x/wq/wk/wv are host-retiled so every DMA line is 1-4KB contiguous;
    blocks 1-3 of x load as flat [128, 2048] tiles (4KB lines); x stays
    SBUF-resident all kernel (~64KB/partition).
  - attention PV matmuls issue two chunks behind the score matmuls so
    the in-order PE queue never waits on the ACT exp chain.
  - RoPE first evacuates PSUM via one ACT copy (bf16), freeing the
    accumulation bank in ~0.6us instead of ~2us and running the 4 DVE
    ops in 2x packed mode.
  - the final outproj issues h0-2 matmuls for two column groups before
    head 3's normalization so the last z chain is hidden.
"""

from contextlib import ExitStack

import ml_dtypes
import numpy as np

import concourse.bass as bass
import concourse.tile as tile
from concourse import bacc, mybir
from concourse.bass_utils import run_bass_kernel_spmd

F32 = mybir.dt.float32
BF16 = mybir.dt.bfloat16
F16 = mybir.dt.float16

BS = 2
SEQ = 2048
DIM = 2048
N_HEADS = 16
N_KV_HEADS = 8
HD = 128
HALF = HD // 2

NCORES = 8
TP = 4                     # tensor-parallel ranks per batch
NH = N_HEADS // TP         # q heads per core = 4
NKV = N_KV_HEADS // TP     # kv heads per core = 2
QB = 512                   # q block (free dim of score matmuls)
KC = 128                   # k chunk (partition dim of P^T tiles)
DC = 128                   # contraction chunk (partitions)
NDC = DIM // DC            # 16
NB = SEQ // QB             # 4 seq blocks
SCALE = 1.0 / np.sqrt(HD)


def _build_nc():
    nc = bacc.Bacc("TRN2", target_bir_lowering=False, debug=False,
                   num_devices=NCORES)
    # host-retiled layouts: [partition][...contiguous cols...]
    xt_d = nc.declare_dram_parameter("xt", [128, NB * NDC * QB], BF16,
                                     isOutput=False)   # [p][j][d][c]
    wq_d = nc.declare_dram_parameter("wq", [128, NDC * NH * HD], BF16,
                                     isOutput=False)   # [p][d][h*128+c]
    wk_d = nc.declare_dram_parameter("wk", [128, NDC * NKV * HD], BF16,
                                     isOutput=False)
    wv_d = nc.declare_dram_parameter("wv", [128, NDC * NKV * HD], BF16,
                                     isOutput=False)
    wo_d = nc.declare_dram_parameter("wo", [NH * HD, DIM], BF16,
                                     isOutput=False)
    cos_d = nc.declare_dram_parameter("cos2", [HD, SEQ], BF16, isOutput=False)
    sin_d = nc.declare_dram_parameter("sins", [HD, SEQ], BF16, isOutput=False)
    msk_d = nc.declare_dram_parameter("maskadd", [KC, KC], BF16,
                                      isOutput=False)
    idn_d = nc.declare_dram_parameter("ident", [KC, KC], BF16, isOutput=False)
    on128_d = nc.declare_dram_parameter("ones128", [128, 1], F16,
                                        isOutput=False)
    on1_d = nc.declare_dram_parameter("ones1", [1, 128], F16, isOutput=False)
    out_d = nc.declare_dram_parameter("out", [SEQ, DIM], BF16, isOutput=True)

    with tile.TileContext(nc) as tc, ExitStack() as ctx:
        wpool = ctx.enter_context(tc.tile_pool(name="weights", bufs=1))
        kvpool = ctx.enter_context(tc.tile_pool(name="kv", bufs=1))
        xpool = ctx.enter_context(tc.tile_pool(name="xt", bufs=1))
        qpool = ctx.enter_context(tc.tile_pool(name="qT", bufs=8))
        ppool = ctx.enter_context(tc.tile_pool(name="pT", bufs=8))
        ospool = ctx.enter_context(tc.tile_pool(name="osb", bufs=8))
        zpool = ctx.enter_context(tc.tile_pool(name="zacc", bufs=3))
        npool = ctx.enter_context(tc.tile_pool(name="norm", bufs=1))
        tpool = ctx.enter_context(tc.tile_pool(name="tmp", bufs=2))
        obpool = ctx.enter_context(tc.tile_pool(name="outb", bufs=8))
        ps_acc = ctx.enter_context(tc.tile_pool(name="ps_acc", bufs=3,
                                                space="PSUM"))
        ps_sc = ctx.enter_context(tc.tile_pool(name="ps_sc", bufs=3,
                                               space="PSUM"))
        ps_att = ctx.enter_context(tc.tile_pool(name="ps_att", bufs=2,
                                                space="PSUM"))

        # ---- persistent weights/constants in SBUF ----
        wq_t = [wpool.tile([128, NH * HD], BF16, tag=f"wq{d}", name=f"wq{d}")
                for d in range(NDC)]
        # wk/wv in 4 quarters (alternating two queues) so the K/V
        # projection matmuls never outrun the weight DMAs
        wk_q = [wpool.tile([128, 4 * NKV * HD], BF16, tag=f"wk{i}",
                       name=f"wk{i}")
                for i in range(4)]
        wv_q = [wpool.tile([128, 4 * NKV * HD], BF16, tag=f"wv{i}",
                       name=f"wv{i}")
                for i in range(4)]
        wk_t = [wk_q[d // 4][:, (d % 4) * NKV * HD:(d % 4 + 1) * NKV * HD]
                for d in range(NDC)]
        wv_t = [wv_q[d // 4][:, (d % 4) * NKV * HD:(d % 4 + 1) * NKV * HD]
                for d in range(NDC)]
        wo_sb = wpool.tile([128, NH * 4 * 512], BF16, tag="wo", name="wo_sb")
        # per-block cos/sin tiles: rope(j) waits only on its own 128KB
        cos_t = [wpool.tile([128, QB], BF16, tag=f"cos{j}", name=f"cos{j}")
                 for j in range(NB)]
        sin_t = [wpool.tile([128, QB], BF16, tag=f"sin{j}", name=f"sin{j}")
                 for j in range(NB)]

        # x resident for the whole kernel.  Block 0: one tile per d-chunk
        # (first matmuls wait only on their own 128KB DMA); blocks 1-3:
        # one tile per 4-chunk group (4KB DMA lines).
        x0_t = [xpool.tile([128, QB], BF16, tag=f"x0_{d}", name=f"x0_{d}")
                for d in range(NDC)]
        xg_t = {(j, g): xpool.tile([128, 4 * QB], BF16, tag=f"x{j}_{g}",
                                   name=f"x{j}_{g}")
                for j in range(1, NB) for g in range(4)}

        def xts(j):
            if j == 0:
                return x0_t
            return [xg_t[j, d // 4][:, (d % 4) * QB:(d % 4 + 1) * QB]
                    for d in range(NDC)]

        # ---- HAM warmup: >3.4us of dummy matmuls flips the PE clock
        # gate to 8/8 before the first real matmuls arrive ----
        warm_w = wpool.tile([128, 128], BF16, tag="warmw", name="warmw")
        warm_x = wpool.tile([128, 64], BF16, tag="warmx", name="warmx")
        nc.vector.memset(warm_w[:], 0.0)
        nc.vector.memset(warm_x[:], 0.0)
        warm_ps = ps_sc.tile([128, 64], F32, tag="sc", name="warm_ps")
        for _ in range(96):
            nc.tensor.matmul(warm_ps[:], warm_w[:], warm_x[:],
                             start=True, stop=True)

        # ---- startup DMAs, 3 queues (scalar/sync/gpsimd are the only
        # DMA-capable engines), block-0-critical bytes first.  The
        # interleaved wq/x0 chunk stream round-robins over all 3 queues
        # so chunk-pairs arrive in consumption order at aggregate BW.
        engs3 = [nc.scalar, nc.sync, nc.gpsimd]
        i = 0
        for d in range(NDC):
            engs3[i % 3].dma_start(wq_t[d][:],
                                   wq_d.ap()[:, d * 512:(d + 1) * 512])
            i += 1
            engs3[i % 3].dma_start(x0_t[d][:],
                                   xt_d.ap()[:, d * QB:(d + 1) * QB])
            i += 1
        # rope(0) gate + attention consts right behind the Q stream
        nc.scalar.dma_start(cos_t[0][:], cos_d.ap()[:, 0:QB])
        nc.scalar.dma_start(sin_t[0][:], sin_d.ap()[:, 0:QB])
        ident = wpool.tile([KC, KC], BF16, tag="ident", name="ident")
        nc.scalar.dma_start(ident[:], idn_d.ap()[:])
        maskA = wpool.tile([KC, KC], BF16, tag="maskA", name="maskA")
        nc.scalar.dma_start(maskA[:], msk_d.ap()[:])
        ones128 = wpool.tile([128, 1], F16, tag="ones128", name="ones128")
        nc.scalar.dma_start(ones128[:], on128_d.ap()[:])
        ones1 = wpool.tile([1, 128], F16, tag="ones1", name="ones1")
        nc.scalar.dma_start(ones1[:], on1_d.ap()[:])
        # wk/wv quarters alternate sync/gpsimd behind x block 0
        for i in range(4):
            eng = nc.sync if i % 2 == 0 else nc.gpsimd
            eng.dma_start(wk_q[i][:],
                          wk_d.ap()[:, i * 1024:(i + 1) * 1024])
        for i in range(4):
            eng = nc.sync if i % 2 == 0 else nc.gpsimd
            eng.dma_start(wv_q[i][:],
                          wv_d.ap()[:, i * 1024:(i + 1) * 1024])
        for j in range(1, NB):
            eng = nc.sync if j % 2 == 1 else nc.gpsimd
            eng.dma_start(cos_t[j][:],
                          cos_d.ap()[:, j * QB:(j + 1) * QB])
            eng.dma_start(sin_t[j][:],
                          sin_d.ap()[:, j * QB:(j + 1) * QB])

        def x_block_dma(j, engs):
            for g in range(4):
                c0 = (j * NDC + 4 * g) * QB
                engs[g % len(engs)].dma_start(
                    xg_t[j, g][:], xt_d.ap()[:, c0:c0 + 4 * QB])

        x_block_dma(1, [nc.sync, nc.gpsimd])
        # wo resident load: [512, 2048] -> [128, (h n c)], 4KB lines
        for h in range(NH):
            nc.scalar.dma_start(
                wo_sb[:, h * 2048:(h + 1) * 2048],
                wo_d.ap()[h * 128:(h + 1) * 128, :])
        x_block_dma(2, [nc.scalar])
        x_block_dma(3, [nc.sync, nc.gpsimd])

        # ---- persistent K^T / V for the whole sequence ----
        kT = [kvpool.tile([128, SEQ], BF16, tag=f"kT{g}", name=f"kT{g}")
              for g in range(NKV)]
        # v_sb columns: [kchunk c][kv head g] -> [:, c*256 + g*128 :+128]
        v_sb = kvpool.tile([128, (SEQ // KC) * NKV * HD], F16, tag="v", name="v_sb")
        assert v_sb.shape[1] == 4096

        def rope(dst, src_ps, cos_t, sin_t):
            """dst = src*cos2 + swap_halves(src)*sins  (dst bf16 SBUF).

            The half-swap muls must read PSUM (DVE only allows a
            partition-base shift when one operand is PSUM); the ACT
            copy in parallel frees the PSUM bank, and the remaining
            cos-mul + add run all-SBUF bf16 in 2x packed mode."""
            nc.vector.tensor_mul(dst[0:64, :], src_ps[64:128, :],
                                 sin_t[0:64, :])
            nc.vector.tensor_mul(dst[64:128, :], src_ps[0:64, :],
                                 sin_t[64:128, :])
            t0 = tpool.tile([128, QB], BF16, tag="ropesrc", name="ropesrc")
            nc.scalar.copy(t0[:], src_ps[:])
            tmp = tpool.tile([128, QB], BF16, tag="ropetmp", name="ropetmp")
            nc.vector.tensor_mul(tmp[:], t0[:], cos_t[:])
            nc.vector.tensor_add(dst[:], dst[:], tmp[:])

        def q_group(j, h, xts_, cos_t, sin_t):
            q_ps = ps_acc.tile([128, QB], F32, tag="acc", name=f"q_ps{j}_{h}")
            for d in range(NDC):
                nc.tensor.matmul(
                    q_ps[:],
                    wq_t[d][:, h * 128:(h + 1) * 128],
                    xts_[d][:], start=(d == 0), stop=(d == NDC - 1))
            qt = qpool.tile([128, QB], BF16, tag="qT", name=f"qt{j}_{h}")
            rope(qt, q_ps, cos_t, sin_t)
            return qt

        def k_group(j, g, xts_, cos_t, sin_t):
            c0 = j * QB
            k_ps = ps_acc.tile([128, QB], F32, tag="acc", name=f"k_ps{j}_{g}")
            for d in range(NDC):
                nc.tensor.matmul(
                    k_ps[:],
                    wk_t[d][:, g * 128:(g + 1) * 128],
                    xts_[d][:], start=(d == 0), stop=(d == NDC - 1))
            rope(kT[g][:, c0:c0 + QB], k_ps, cos_t, sin_t)

        def v_group(j, m, xts_):
            v_ps = ps_acc.tile([128, NKV * HD], F32, tag="acc",
                               name=f"v_ps{j}_{m}")
            for d in range(NDC):
                nc.tensor.matmul(
                    v_ps[:],
                    xts_[d][:, m * 128:(m + 1) * 128],
                    wv_t[d][:],
                    start=(d == 0), stop=(d == NDC - 1))
            kc = 4 * j + m
            with nc.allow_low_precision(reason="V in fp16 (11-bit) is plenty"):
                nc.scalar.copy(v_sb[:, kc * 256:(kc + 1) * 256], v_ps[:])

        def wo_ap(n, h):
            return wo_sb[:, h * 2048 + n * 512: h * 2048 + (n + 1) * 512]

        def attn_head(j, nkc, qT, h):
            """Scores/exp/PV for one head; PV issues two chunks behind
            the score matmuls so the in-order PE queue never waits on
            the ACT exp chain."""
            g = h // 2
            o_ps = ps_att.tile([128, QB], F32, tag="att", name=f"o_ps{j}_{h}")
            zacc = zpool.tile([128, QB], F16, tag="zacc",
                              name=f"zacc{j}_{h}")
            pts = [None] * nkc
            offs = [max(0, (kc - 4 * j) * 128) for kc in range(nkc)]

            def issue_score(kc):
                off = offs[kc]
                sc_ps = ps_sc.tile([128, QB], F32, tag="sc",
                                   name=f"sc{j}_{h}_{kc}")
                if kc >= 4 * j:
                    # additive causal mask: -1e9 above the diagonal, via
                    # a tiny identity-stationary matmul into the bank
                    nc.tensor.matmul(sc_ps[:, off:off + KC], ident[:],
                                     maskA[:], start=True, stop=False)
                    nc.tensor.matmul(sc_ps[:, off:QB],
                                     kT[g][:, kc * 128:(kc + 1) * 128],
                                     qT[h][:, off:QB], start=False, stop=True)
                else:
                    nc.tensor.matmul(sc_ps[:, off:QB],
                                     kT[g][:, kc * 128:(kc + 1) * 128],
                                     qT[h][:, off:QB], start=True, stop=True)
                pt = ppool.tile([128, QB], F16, tag="pT",
                                name=f"pt{j}_{h}_{kc}")
                nc.scalar.activation(pt[:, off:QB], sc_ps[:, off:QB],
                                     mybir.ActivationFunctionType.Exp,
                                     scale=float(SCALE))
                with nc.allow_low_precision(
                        reason="softmax z accum in fp16 (11-bit) is plenty"):
                    if kc == 0:
                        nc.vector.tensor_copy(zacc[:], pt[:])
                    else:
                        nc.vector.tensor_add(zacc[:, off:QB],
                                             zacc[:, off:QB],
                                             pt[:, off:QB])
                pts[kc] = pt

            def issue_pv(kc):
                off = offs[kc]
                nc.tensor.matmul(o_ps[:, off:QB],
                                 v_sb[:, kc * 256 + g * 128:
                                      kc * 256 + (g + 1) * 128],
                                 pts[kc][:, off:QB], start=(kc == 0),
                                 stop=(kc == nkc - 1))

            issue_score(0)
            issue_score(1)
            for kc in range(2, nkc):
                issue_score(kc)
                issue_pv(kc - 2)
            issue_pv(nkc - 2)
            issue_pv(nkc - 1)
            # stage unnormalized O' now (frees the PSUM bank quickly); the
            # z finalize is issued separately (z_fin) one head later
            o_sb = ospool.tile([128, QB], BF16, tag="osb", name=f"o_sb{j}_{h}")
            nc.scalar.copy(o_sb[:], o_ps[:])
            return (o_sb, zacc)

        def z_fin(j, h, zacc):
            # z row = ones^T @ zacc (partition reduce); 1/z = exp(-ln z)
            # on ACT (Ln+Exp share a table set) -- no DMA round-trip.
            z_ps = ps_sc.tile([1, QB], F32, tag="sc", name=f"z_ps{j}_{h}")
            nc.tensor.matmul(z_ps[:], ones128[:], zacc[:],
                             start=True, stop=True)
            lnz = npool.tile([1, QB], F32, tag="lnz", bufs=4,
                             name=f"lnz{j}_{h}")
            nc.scalar.activation(lnz[:], z_ps[:],
                                 mybir.ActivationFunctionType.Ln)
            rz = npool.tile([1, QB], F16, tag="rz", bufs=6,
                            name=f"rz{j}_{h}")
            with nc.allow_low_precision(
                    reason="1/z in fp16 (11-bit mantissa) is plenty"):
                nc.scalar.activation(rz[:], lnz[:],
                                     mybir.ActivationFunctionType.Exp,
                                     scale=-1.0)
            return rz

        def norm_head(j, h, o_sb, rz):
            # o_sb *= broadcast(1/z) (in place)
            zb_ps = ps_sc.tile([128, QB], F32, tag="sc", name=f"zb{j}_{h}")
            nc.tensor.matmul(zb_ps[:], ones1[:], rz[:], start=True, stop=True)
            nc.vector.tensor_mul(o_sb[:], o_sb[:], zb_ps[:])

        def op_group(j, n, mp, oT, pool, tag, heads, start, stop, evac):
            """Issue outproj matmuls for heads `heads` of column group
            (n, mp) into 2 PSUM banks from `pool`; returns the banks."""
            op_ps = [pool.tile([128, 512], F32, tag=tag,
                               name=f"op{j}_{n}_{mp}_{m}")
                     for m in range(2)]
            return op_cont(j, n, mp, oT, op_ps, heads, start, stop, evac)

        def op_cont(j, n, mp, oT, op_ps, heads, start, stop, evac):
            c0 = j * QB
            for h in heads:
                for mi in range(2):
                    m = 2 * mp + mi
                    nc.tensor.matmul(
                        op_ps[mi][:],
                        oT[h][:, m * 128:(m + 1) * 128],
                        wo_ap(n, h),
                        start=(h == heads[0] and start),
                        stop=(h == heads[-1] and stop))
            if evac:
                for mi in range(2):
                    m = 2 * mp + mi
                    ob = obpool.tile([128, 512], BF16, tag="ob",
                                     name=f"ob{j}_{n}_{m}")
                    # split PSUM->SBUF evacuations between ACT and DVE
                    if mi == 0:
                        nc.scalar.copy(ob[:], op_ps[mi][:])
                    else:
                        nc.vector.tensor_copy(ob[:], op_ps[mi][:])
                    nc.sync.dma_start(
                        out_d.ap()[c0 + m * 128: c0 + (m + 1) * 128,
                                   n * 512:(n + 1) * 512], ob[:])
            return op_ps

        def outproj_block(j, oT, rzs, skip_norm=(), ns=(0, 1, 2, 3)):
            for h in range(NH):
                if h not in skip_norm:
                    norm_head(j, h, oT[h], rzs[h])
            for n in ns:
                for mp in range(2):
                    op_group(j, n, mp, oT, ps_acc, "acc", list(range(NH)),
                             True, True, True)

        # ---- software pipeline ----
        # Block 0 QKV: all 4 q-head accumulations interleaved per
        # d-chunk so the PE consumes each x/wq chunk-pair as it lands.
        cos0 = cos_t[0][:]
        sin0 = sin_t[0][:]
        q_ps0 = [ps_acc.tile([128, QB], F32, tag="acc", name=f"q_ps0_{h}")
                 for h in range(3)]
        q_ps0.append(ps_att.tile([128, QB], F32, tag="att", name="q_ps0_3"))
        for d in range(NDC):
            for h in range(NH):
                nc.tensor.matmul(
                    q_ps0[h][:],
                    wq_t[d][:, h * 128:(h + 1) * 128],
                    x0_t[d][:], start=(d == 0), stop=(d == NDC - 1))
        qT_cur = []
        for h in range(NH):
            qt = qpool.tile([128, QB], BF16, tag="qT", name=f"qt0_{h}")
            rope(qt, q_ps0[h], cos0, sin0)
            qT_cur.append(qt)
        for g in range(NKV):
            k_group(0, g, x0_t, cos0, sin0)
        for m in range(4):
            v_group(0, m, x0_t)

        LAST = NB - 1
        oT3, rz3 = [], []
        for j in range(NB - 1):
            nkc = 4 * (j + 1)
            xtsn = xts(j + 1)
            cosn = cos_t[j + 1][:]
            sinn = sin_t[j + 1][:]
            oT_cur = [attn_head(j, nkc, qT_cur, 0),
                      attn_head(j, nkc, qT_cur, 1)]
            rz_cur = [z_fin(j, 0, oT_cur[0][1])]
            qT_next = [q_group(j + 1, 0, xtsn, cosn, sinn)]
            oT_cur.append(attn_head(j, nkc, qT_cur, 2))
            rz_cur.append(z_fin(j, 1, oT_cur[1][1]))
            qT_next.append(q_group(j + 1, 1, xtsn, cosn, sinn))
            oT_cur.append(attn_head(j, nkc, qT_cur, 3))
            rz_cur.append(z_fin(j, 2, oT_cur[2][1]))
            qT_next.append(q_group(j + 1, 2, xtsn, cosn, sinn))
            qT_next.append(q_group(j + 1, 3, xtsn, cosn, sinn))
            rz_cur.append(z_fin(j, 3, oT_cur[3][1]))
            for g in range(NKV):
                k_group(j + 1, g, xtsn, cosn, sinn)
            for m in range(4):
                v_group(j + 1, m, xtsn)
            qT_cur = qT_next
            oT_sb = [o for o, _ in oT_cur]
            if j == NB - 2:
                # interleave the last block's heads with outproj(j) halves
                # so the exp-bound chains overlap dense matmul work
                oT3.append(attn_head(LAST, 4 * NB, qT_cur, 0))
                outproj_block(j, oT_sb, rz_cur, ns=(0, 1))
                rz3.append(z_fin(LAST, 0, oT3[0][1]))
                oT3.append(attn_head(LAST, 4 * NB, qT_cur, 1))
                outproj_block(j, oT_sb, rz_cur,
                              skip_norm=(0, 1, 2, 3), ns=(2, 3))
                rz3.append(z_fin(LAST, 1, oT3[1][1]))
                oT3.append(attn_head(LAST, 4 * NB, qT_cur, 2))
                rz3.append(z_fin(LAST, 2, oT3[2][1]))
            else:
                outproj_block(j, oT_sb, rz_cur)
        for h in range(3):
            norm_head(LAST, h, oT3[h][0], rz3[h])
        oT3.append(attn_head(LAST, 4 * NB, qT_cur, 3))
        rz3.append(z_fin(LAST, 3, oT3[3][1]))

        # ---- final outproj: issue h0-2 matmuls for two column groups
        # BEFORE head 3's normalization so the PE queue has ~2.6us of
        # dense work covering the last z chain ----
        oT_f = [o for o, _ in oT3]
        gA = op_group(LAST, 0, 0, oT_f, ps_acc, "acc", [0, 1, 2],
                      True, False, False)
        gB = op_group(LAST, 0, 1, oT_f, ps_att, "att", [0, 1, 2],
                      True, False, False)
        norm_head(LAST, 3, oT_f[3], rz3[3])
        op_cont(LAST, 0, 0, oT_f, gA, [3], False, True, True)
        op_cont(LAST, 0, 1, oT_f, gB, [3], False, True, True)
        for n in (1, 2, 3):
            for mp in range(2):
                op_group(LAST, n, mp, oT_f, ps_acc, "acc", list(range(NH)),
                         True, True, True)

    nc.compile()
    return nc


_NC_CACHE = None


def _get_nc():
    global _NC_CACHE
    if _NC_CACHE is None:
        _NC_CACHE = _build_nc()
    return _NC_CACHE


def _host_prep(inputs):
    """Build the 8 per-core input maps from the full problem inputs."""
    hs = np.asarray(inputs["hidden_state"], dtype=np.float32)
    cos = np.asarray(inputs["freq_cos"], dtype=np.float32)[0, :, 0, :]  # [S,64]
    sin = np.asarray(inputs["freq_sin"], dtype=np.float32)[0, :, 0, :]
    wq = np.asarray(inputs["wq"], dtype=np.float32)
    wk = np.asarray(inputs["wk"], dtype=np.float32)
    wv = np.asarray(inputs["wv"], dtype=np.float32)
    wo = np.asarray(inputs["wo"], dtype=np.float32)

    perm = np.concatenate([np.arange(0, HD, 2), np.arange(1, HD, 2)])  # [128]

    cos2 = np.empty((HD, SEQ), dtype=np.float32)
    sins = np.empty((HD, SEQ), dtype=np.float32)
    cos2[:HALF] = cos.T
    cos2[HALF:] = cos.T
    sins[:HALF] = -sin.T
    sins[HALF:] = sin.T
    cos2 = cos2.astype(ml_dtypes.bfloat16)
    sins = sins.astype(ml_dtypes.bfloat16)

    ki = np.arange(KC)
    # additive causal mask for diagonal chunks: key k > query c -> -1e9
    maskadd = np.where(ki[:, None] > ki[None, :], -1e9,
                       0.0).astype(ml_dtypes.bfloat16)
    ident = np.eye(KC, dtype=ml_dtypes.bfloat16)
    ones128 = np.ones((128, 1), dtype=np.float16)
    ones1 = np.ones((1, 128), dtype=np.float16)

    def tile_pdc(w):
        # [2048, C] -> [128, 16*C]: row p holds chunks d=0..15 contiguously
        c = w.shape[1]
        return np.ascontiguousarray(
            w.reshape(NDC, 128, c).transpose(1, 0, 2).reshape(128, NDC * c))

    # x^T -> [p][j][d][c] so block-j 4-chunk groups are 4KB-contiguous
    xTs = []
    for b in range(BS):
        xT = hs[b].T.astype(ml_dtypes.bfloat16)          # [dim, seq]
        x4 = xT.reshape(NDC, 128, NB, QB).transpose(1, 2, 0, 3)
        xTs.append(np.ascontiguousarray(x4.reshape(128, NB * NDC * QB)))

    in_maps = []
    for c in range(NCORES):
        b, r = divmod(c, TP)
        qcols = np.concatenate(
            [(4 * r + h) * HD + perm for h in range(NH)])
        kcols = np.concatenate(
            [(NKV * r + g) * HD + perm for g in range(NKV)])
        vcols = np.concatenate(
            [(NKV * r + g) * HD + np.arange(HD) for g in range(NKV)])
        worows = np.concatenate(
            [(4 * r + h) * HD + np.arange(HD) for h in range(NH)])
        in_maps.append({
            "xt": xTs[b],
            "wq": tile_pdc(wq[:, qcols].astype(ml_dtypes.bfloat16)),
            "wk": tile_pdc(wk[:, kcols].astype(ml_dtypes.bfloat16)),
            "wv": tile_pdc(wv[:, vcols].astype(ml_dtypes.bfloat16)),
            "wo": np.ascontiguousarray(wo[worows, :]).astype(ml_dtypes.bfloat16),
            "cos2": cos2,
            "sins": sins,
            "maskadd": maskadd,
            "ident": ident,
            "ones128": ones128,
            "ones1": ones1,
        })
    return in_maps


def _run(inputs, trace=False, **trace_kwargs):
    nc = _get_nc()
    in_maps = _host_prep(inputs)
    res = run_bass_kernel_spmd(nc, in_maps, list(range(NCORES)),
                               trace=trace, **trace_kwargs)
    out = np.zeros((BS, SEQ, DIM), dtype=np.float32)
    for c in range(NCORES):
        out[c // TP] += np.asarray(res.results[c]["out"], dtype=np.float32)
    return out, res


def kernel(**inputs) -> np.ndarray:
    out, _ = _run(inputs, trace=False)
    return out


# revision 13
# speedup vs baseline: 1.2442x; 1.2442x over previous
"""GQA attention (bs=2, seq=2048, dim=2048, 16 q-heads / 8 kv-heads, hd=128)
on 8 Trainium2 NeuronCores.

Sharding: 2-way data parallel (batch) x 4-way tensor parallel (heads, kv
groups intact).  Core c handles batch c//4 and q-heads [4*(c%4), 4*(c%4)+4)
(kv-heads [2*(c%4), 2*(c%4)+2)).  Each core computes a partial output
projection (row-split wo); the all-reduce over the 4 TP ranks is done on the
host while gathering (bf16 partials summed in f32).

Device kernel (per core):
  - all inputs bf16 (weights, x^T) -> FWL-eligible stationaries, half DMA.
  - host supplies X^T (so `dim` lands on partitions for every projection)
    and rotate-half permuted wq/wk, so RoPE is 4 DVE ops per tile.
  - scores are computed transposed (P^T[k, q]) which makes PV and the
    output projection transpose-free.
  - causal masking is additive: a [128,128] -1e9 strictly-lower matrix is
    accumulated into the scores PSUM bank by a tiny N=128 matmul
    (identity stationary) before the score matmul, so exp() produces
    exact zeros and the DVE mask multiply disappears from the
    exp->PV chain.
  - softmax row-sums: P^T chunks are accumulated into a [128, QB] fp16
    SBUF tile by DVE adds; one all-ones [128,1] matmul per head-block
    reduces over partitions; 1/z = exp(-ln(z)) on the scalar engine
    (Ln and Exp share one ACT table set), avoiding any DMA round-trip;
    a [1,128] ones matmul broadcasts 1/z back to 128 partitions for the
    DVE normalization multiply.

Perf notes (vs the first working version, 312.4us -> target ~270us):
  - 96 warmup matmuls (>3.4us busy) so the PE HAM clock-gate opens at
    ~3.4us instead of 50us; previously the whole DMA-fed ramp ran at
    1.2GHz.
  - startup DMAs spread over 4 engine queues (scalar/vector for wq,
    sync/gpsimd for x block 0) and block-0 Q accumulates all 4 heads
    per d-chunk, so the PE consumes each 2x128KB chunk-pair (863ns) at
    the pace DMA delivers it.
  - x/wq/wk/wv are host-retiled so every DMA line is 1-4KB contiguous;
    blocks 1-3 of x load as flat [128, 2048] tiles (4KB lines); x stays
    SBUF-resident all kernel (~64KB/partition).
  - attention PV matmuls issue two chunks behind the score matmuls so
    the in-order PE queue never waits on the ACT exp chain.
  - RoPE first evacuates PSUM via one ACT copy (bf16), freeing the
    accumulation bank in ~0.6us instead of ~2us and running the 4 DVE
    ops in 2x packed mode.
  - the final outproj issues h0-2 matmuls for two column groups before
    head 3's normalization so the last z chain is hidden.
"""

from contextlib import ExitStack

import ml_dtypes
import numpy as np

import concourse.bass as bass
import concourse.tile as tile
from concourse import bacc, mybir
from concourse.bass_utils import run_bass_kernel_spmd

F32 = mybir.dt.float32
BF16 = mybir.dt.bfloat16
F16 = mybir.dt.float16

BS = 2
SEQ = 2048
DIM = 2048
N_HEADS = 16
N_KV_HEADS = 8
HD = 128
HALF = HD // 2

NCORES = 8
TP = 4                     # tensor-parallel ranks per batch
NH = N_HEADS // TP         # q heads per core = 4
NKV = N_KV_HEADS // TP     # kv heads per core = 2
QB = 512                   # q block (free dim of score matmuls)
KC = 128                   # k chunk (partition dim of P^T tiles)
DC = 128                   # contraction chunk (partitions)
NDC = DIM // DC            # 16
NB = SEQ // QB             # 4 seq blocks
SCALE = 1.0 / np.sqrt(HD)


def _build_nc():
    nc = bacc.Bacc("TRN2", target_bir_lowering=False, debug=False,
                   num_devices=NCORES)
    # host-retiled layouts: [partition][...contiguous cols...]
    xt_d = nc.declare_dram_parameter("xt", [128, NB * NDC * QB], BF16,
                                     isOutput=False)   # [p][j][d][c]
    wq_d = nc.declare_dram_parameter("wq", [128, NDC * NH * HD], BF16,
                                     isOutput=False)   # [p][d][h*128+c]
    wk_d = nc.declare_dram_parameter("wk", [128, NDC * NKV * HD], BF16,
                                     isOutput=False)
    wv_d = nc.declare_dram_parameter("wv", [128, NDC * NKV * HD], BF16,
                                     isOutput=False)
    wo_d = nc.declare_dram_parameter("wo", [NH * HD, DIM], BF16,
                                     isOutput=False)
    cos_d = nc.declare_dram_parameter("cos2", [HD, SEQ], BF16, isOutput=False)
    sin_d = nc.declare_dram_parameter("sins", [HD, SEQ], BF16, isOutput=False)
    msk_d = nc.declare_dram_parameter("maskadd", [KC, KC], BF16,
                                      isOutput=False)
    idn_d = nc.declare_dram_parameter("ident", [KC, KC], BF16, isOutput=False)
    on128_d = nc.declare_dram_parameter("ones128", [128, 1], F16,
                                        isOutput=False)
    on1_d = nc.declare_dram_parameter("ones1", [1, 128], F16, isOutput=False)
    out_d = nc.declare_dram_parameter("out", [SEQ, DIM], BF16, isOutput=True)

    with tile.TileContext(nc) as tc, ExitStack() as ctx:
        wpool = ctx.enter_context(tc.tile_pool(name="weights", bufs=1))
        kvpool = ctx.enter_context(tc.tile_pool(name="kv", bufs=1))
        xpool = ctx.enter_context(tc.tile_pool(name="xt", bufs=1))
        qpool = ctx.enter_context(tc.tile_pool(name="qT", bufs=8))
        ppool = ctx.enter_context(tc.tile_pool(name="pT", bufs=8))
        ospool = ctx.enter_context(tc.tile_pool(name="osb", bufs=8))
        zpool = ctx.enter_context(tc.tile_pool(name="zacc", bufs=3))
        npool = ctx.enter_context(tc.tile_pool(name="norm", bufs=1))
        tpool = ctx.enter_context(tc.tile_pool(name="tmp", bufs=2))
        obpool = ctx.enter_context(tc.tile_pool(name="outb", bufs=8))
        ps_acc = ctx.enter_context(tc.tile_pool(name="ps_acc", bufs=3,
                                                space="PSUM"))
        ps_sc = ctx.enter_context(tc.tile_pool(name="ps_sc", bufs=3,
                                               space="PSUM"))
        ps_att = ctx.enter_context(tc.tile_pool(name="ps_att", bufs=2,
                                                space="PSUM"))

        # ---- persistent weights/constants in SBUF ----
        wq_t = [wpool.tile([128, NH * HD], BF16, tag=f"wq{d}", name=f"wq{d}")
                for d in range(NDC)]
        # wk/wv in 4 quarters (alternating two queues) so the K/V
        # projection matmuls never outrun the weight DMAs
        wk_q = [wpool.tile([128, 4 * NKV * HD], BF16, tag=f"wk{i}",
                       name=f"wk{i}")
                for i in range(4)]
        wv_q = [wpool.tile([128, 4 * NKV * HD], BF16, tag=f"wv{i}",
                       name=f"wv{i}")
                for i in range(4)]
        wk_t = [wk_q[d // 4][:, (d % 4) * NKV * HD:(d % 4 + 1) * NKV * HD]
                for d in range(NDC)]
        wv_t = [wv_q[d // 4][:, (d % 4) * NKV * HD:(d % 4 + 1) * NKV * HD]
                for d in range(NDC)]
        wo_sb = wpool.tile([128, NH * 4 * 512], BF16, tag="wo", name="wo_sb")
        # per-block cos/sin tiles: rope(j) waits only on its own 128KB
        cos_t = [wpool.tile([128, QB], BF16, tag=f"cos{j}", name=f"cos{j}")
                 for j in range(NB)]
        sin_t = [wpool.tile([128, QB], BF16, tag=f"sin{j}", name=f"sin{j}")
                 for j in range(NB)]

        # x resident for the whole kernel.  Block 0: one tile per d-chunk
        # (first matmuls wait only on their own 128KB DMA); blocks 1-3:
        # one tile per 4-chunk group (4KB DMA lines).
        x0_t = [xpool.tile([128, QB], BF16, tag=f"x0_{d}", name=f"x0_{d}")
                for d in range(NDC)]
        xg_t = {(j, g): xpool.tile([128, 4 * QB], BF16, tag=f"x{j}_{g}",
                                   name=f"x{j}_{g}")
                for j in range(1, NB) for g in range(4)}

        def xts(j):
            if j == 0:
                return x0_t
            return [xg_t[j, d // 4][:, (d % 4) * QB:(d % 4 + 1) * QB]
                    for d in range(NDC)]

        # ---- HAM warmup: >3.4us of dummy matmuls flips the PE clock
        # gate to 8/8 before the first real matmuls arrive ----
        warm_w = wpool.tile([128, 128], BF16, tag="warmw", name="warmw")
        warm_x = wpool.tile([128, 64], BF16, tag="warmx", name="warmx")
        nc.vector.memset(warm_w[:], 0.0)
        nc.vector.memset(warm_x[:], 0.0)
        warm_ps = ps_sc.tile([128, 64], F32, tag="sc", name="warm_ps")
        for _ in range(96):
            nc.tensor.matmul(warm_ps[:], warm_w[:], warm_x[:],
                             start=True, stop=True)

        # ---- startup DMAs, 3 queues (scalar/sync/gpsimd are the only
        # DMA-capable engines), block-0-critical bytes first.  The
        # interleaved wq/x0 chunk stream round-robins over all 3 queues
        # so chunk-pairs arrive in consumption order at aggregate BW.
        engs3 = [nc.scalar, nc.sync, nc.gpsimd]
        i = 0
        for d in range(NDC):
            engs3[i % 3].dma_start(wq_t[d][:],
                                   wq_d.ap()[:, d * 512:(d + 1) * 512])
            i += 1
            engs3[i % 3].dma_start(x0_t[d][:],
                                   xt_d.ap()[:, d * QB:(d + 1) * QB])
            i += 1
        # rope(0) gate + attention consts right behind the Q stream
        nc.scalar.dma_start(cos_t[0][:], cos_d.ap()[:, 0:QB])
        nc.scalar.dma_start(sin_t[0][:], sin_d.ap()[:, 0:QB])
        ident = wpool.tile([KC, KC], BF16, tag="ident", name="ident")
        nc.scalar.dma_start(ident[:], idn_d.ap()[:])
        maskA = wpool.tile([KC, KC], BF16, tag="maskA", name="maskA")
        nc.scalar.dma_start(maskA[:], msk_d.ap()[:])
        ones128 = wpool.tile([128, 1], F16, tag="ones128", name="ones128")
        nc.scalar.dma_start(ones128[:], on128_d.ap()[:])
        ones1 = wpool.tile([1, 128], F16, tag="ones1", name="ones1")
        nc.scalar.dma_start(ones1[:], on1_d.ap()[:])
        # wk/wv quarters alternate sync/gpsimd behind x block 0
        for i in range(4):
            eng = nc.sync if i % 2 == 0 else nc.gpsimd
            eng.dma_start(wk_q[i][:],
                          wk_d.ap()[:, i * 1024:(i + 1) * 1024])
        for i in range(4):
            eng = nc.sync if i % 2 == 0 else nc.gpsimd
            eng.dma_start(wv_q[i][:],
                          wv_d.ap()[:, i * 1024:(i + 1) * 1024])
        for j in range(1, NB):
            eng = nc.sync if j % 2 == 1 else nc.gpsimd
            eng.dma_start(cos_t[j][:],
                          cos_d.ap()[:, j * QB:(j + 1) * QB])
            eng.dma_start(sin_t[j][:],
                          sin_d.ap()[:, j * QB:(j + 1) * QB])

        def x_block_dma(j, engs):
            for g in range(4):
                c0 = (j * NDC + 4 * g) * QB
                engs[g % len(engs)].dma_start(
                    xg_t[j, g][:], xt_d.ap()[:, c0:c0 + 4 * QB])

        x_block_dma(1, [nc.sync, nc.gpsimd])
        # wo resident load: [512, 2048] -> [128, (h n c)], 4KB lines
        for h in range(NH):
            nc.scalar.dma_start(
                wo_sb[:, h * 2048:(h + 1) * 2048],
                wo_d.ap()[h * 128:(h + 1) * 128, :])
        x_block_dma(2, [nc.scalar])
        x_block_dma(3, [nc.sync, nc.gpsimd])

        # ---- persistent K^T / V for the whole sequence ----
        kT = [kvpool.tile([128, SEQ], BF16, tag=f"kT{g}", name=f"kT{g}")
              for g in range(NKV)]
        # v_sb columns: [kchunk c][kv head g] -> [:, c*256 + g*128 :+128]
        v_sb = kvpool.tile([128, (SEQ // KC) * NKV * HD], F16, tag="v", name="v_sb")
        assert v_sb.shape[1] == 4096

        def rope(dst, src_ps, cos_t, sin_t):
            """dst = src*cos2 + swap_halves(src)*sins  (dst bf16 SBUF).

            The half-swap muls must read PSUM (DVE only allows a
            partition-base shift when one operand is PSUM); the ACT
            copy in parallel frees the PSUM bank, and the remaining
            cos-mul + add run all-SBUF bf16 in 2x packed mode."""
            nc.vector.tensor_mul(dst[0:64, :], src_ps[64:128, :],
                                 sin_t[0:64, :])
            nc.vector.tensor_mul(dst[64:128, :], src_ps[0:64, :],
                                 sin_t[64:128, :])
            t0 = tpool.tile([128, QB], BF16, tag="ropesrc", name="ropesrc")
            nc.scalar.copy(t0[:], src_ps[:])
            tmp = tpool.tile([128, QB], BF16, tag="ropetmp", name="ropetmp")
            nc.vector.tensor_mul(tmp[:], t0[:], cos_t[:])
            nc.vector.tensor_add(dst[:], dst[:], tmp[:])

        def q_group(j, h, xts_, cos_t, sin_t):
            q_ps = ps_acc.tile([128, QB], F32, tag="acc", name=f"q_ps{j}_{h}")
            for d in range(NDC):
                nc.tensor.matmul(
                    q_ps[:],
                    wq_t[d][:, h * 128:(h + 1) * 128],
                    xts_[d][:], start=(d == 0), stop=(d == NDC - 1))
            qt = qpool.tile([128, QB], BF16, tag="qT", name=f"qt{j}_{h}")
            rope(qt, q_ps, cos_t, sin_t)
            return qt

        def k_group(j, g, xts_, cos_t, sin_t):
            c0 = j * QB
            k_ps = ps_acc.tile([128, QB], F32, tag="acc", name=f"k_ps{j}_{g}")
            for d in range(NDC):
                nc.tensor.matmul(
                    k_ps[:],
                    wk_t[d][:, g * 128:(g + 1) * 128],
                    xts_[d][:], start=(d == 0), stop=(d == NDC - 1))
            rope(kT[g][:, c0:c0 + QB], k_ps, cos_t, sin_t)

        def v_group(j, m, xts_):
            v_ps = ps_acc.tile([128, NKV * HD], F32, tag="acc",
                               name=f"v_ps{j}_{m}")
            for d in range(NDC):
                nc.tensor.matmul(
                    v_ps[:],
                    xts_[d][:, m * 128:(m + 1) * 128],
                    wv_t[d][:],
                    start=(d == 0), stop=(d == NDC - 1))
            kc = 4 * j + m
            with nc.allow_low_precision(reason="V in fp16 (11-bit) is plenty"):
                nc.scalar.copy(v_sb[:, kc * 256:(kc + 1) * 256], v_ps[:])

        def wo_ap(n, h):
            return wo_sb[:, h * 2048 + n * 512: h * 2048 + (n + 1) * 512]

        def attn_head(j, nkc, qT, h):
            """Scores/exp/PV for one head; PV issues two chunks behind
            the score matmuls so the in-order PE queue never waits on
            the ACT exp chain."""
            g = h // 2
            o_ps = ps_att.tile([128, QB], F32, tag="att", name=f"o_ps{j}_{h}")
            zacc = zpool.tile([128, QB], F16, tag="zacc",
                              name=f"zacc{j}_{h}")
            pts = [None] * nkc
            offs = [max(0, (kc - 4 * j) * 128) for kc in range(nkc)]

            def issue_score(kc):
                off = offs[kc]
                sc_ps = ps_sc.tile([128, QB], F32, tag="sc",
                                   name=f"sc{j}_{h}_{kc}")
                if kc >= 4 * j:
                    # additive causal mask: -1e9 above the diagonal, via
                    # a tiny identity-stationary matmul into the bank
                    nc.tensor.matmul(sc_ps[:, off:off + KC], ident[:],
                                     maskA[:], start=True, stop=False)
                    nc.tensor.matmul(sc_ps[:, off:QB],
                                     kT[g][:, kc * 128:(kc + 1) * 128],
                                     qT[h][:, off:QB], start=False, stop=True)
                else:
                    nc.tensor.matmul(sc_ps[:, off:QB],
                                     kT[g][:, kc * 128:(kc + 1) * 128],
                                     qT[h][:, off:QB], start=True, stop=True)
                pt = ppool.tile([128, QB], F16, tag="pT",
                                name=f"pt{j}_{h}_{kc}")
                nc.scalar.activation(pt[:, off:QB], sc_ps[:, off:QB],
                                     mybir.ActivationFunctionType.Exp,
                                     scale=float(SCALE))
                with nc.allow_low_precision(
                        reason="softmax z accum in fp16 (11-bit) is plenty"):
                    if kc == 0:
                        nc.vector.tensor_copy(zacc[:], pt[:])
                    else:
                        nc.vector.tensor_add(zacc[:, off:QB],
                                             zacc[:, off:QB],
                                             pt[:, off:QB])
                pts[kc] = pt

            def issue_pv(kc):
                off = offs[kc]
                nc.tensor.matmul(o_ps[:, off:QB],
                                 v_sb[:, kc * 256 + g * 128:
                                      kc * 256 + (g + 1) * 128],
                                 pts[kc][:, off:QB], start=(kc == 0),
                                 stop=(kc == nkc - 1))

            issue_score(0)
            issue_score(1)
            for kc in range(2, nkc):
                issue_score(kc)
                issue_pv(kc - 2)
            issue_pv(nkc - 2)
            issue_pv(nkc - 1)
            # stage unnormalized O' now (frees the PSUM bank quickly); the
            # z finalize is issued separately (z_fin) one head later
            o_sb = ospool.tile([128, QB], BF16, tag="osb", name=f"o_sb{j}_{h}")
            nc.scalar.copy(o_sb[:], o_ps[:])
            return (o_sb, zacc)

        def z_fin(j, h, zacc):
            # z row = ones^T @ zacc (partition reduce), then reshape the z
            # row to [128,4] so the reciprocal runs on all 128 DVE lanes;
            # reshape hops ride the GPSIMD queue, which is idle once the
            # x loads finish (sync carries output writes and would add
            # multi-us queueing latency at the tail)
            z_ps = ps_sc.tile([1, QB], F32, tag="sc", name=f"z_ps{j}_{h}")
            nc.tensor.matmul(z_ps[:], ones128[:], zacc[:],
                             start=True, stop=True)
            z_sb = npool.tile([1, QB], F32, tag="z", bufs=4,
                              name=f"z_sb{j}_{h}")
            nc.scalar.copy(z_sb[:], z_ps[:])
            zc = npool.tile([128, QB // 128], F32, tag="zc", bufs=4,
                            name=f"zc{j}_{h}")
            nc.gpsimd.dma_start(zc[:], z_sb[:])
            rzc = npool.tile([128, QB // 128], F16, tag="rzc", bufs=4,
                             name=f"rzc{j}_{h}")
            with nc.allow_low_precision(
                    reason="1/z in fp16 (11-bit mantissa) is plenty"):
                nc.vector.reciprocal(rzc[:], zc[:])
            rz = npool.tile([1, QB], F16, tag="rz", bufs=6,
                            name=f"rz{j}_{h}")
            nc.gpsimd.dma_start(rz[:], rzc[:])
            return rz

        def norm_head(j, h, o_sb, rz):
            # o_sb *= broadcast(1/z) (in place)
            zb_ps = ps_sc.tile([128, QB], F32, tag="sc", name=f"zb{j}_{h}")
            nc.tensor.matmul(zb_ps[:], ones1[:], rz[:], start=True, stop=True)
            nc.vector.tensor_mul(o_sb[:], o_sb[:], zb_ps[:])

        def op_group(j, n, mp, oT, pool, tag, heads, start, stop, evac):
            """Issue outproj matmuls for heads `heads` of column group
            (n, mp) into 2 PSUM banks from `pool`; returns the banks."""
            op_ps = [pool.tile([128, 512], F32, tag=tag,
                               name=f"op{j}_{n}_{mp}_{m}")
                     for m in range(2)]
            return op_cont(j, n, mp, oT, op_ps, heads, start, stop, evac)

        def op_cont(j, n, mp, oT, op_ps, heads, start, stop, evac):
            c0 = j * QB
            for h in heads:
                for mi in range(2):
                    m = 2 * mp + mi
                    nc.tensor.matmul(
                        op_ps[mi][:],
                        oT[h][:, m * 128:(m + 1) * 128],
                        wo_ap(n, h),
                        start=(h == heads[0] and start),
                        stop=(h == heads[-1] and stop))
            if evac:
                for mi in range(2):
                    m = 2 * mp + mi
                    ob = obpool.tile([128, 512], BF16, tag="ob",
                                     name=f"ob{j}_{n}_{m}")
                    # split PSUM->SBUF evacuations between ACT and DVE
                    if mi == 0:
                        nc.scalar.copy(ob[:], op_ps[mi][:])
                    else:
                        nc.vector.tensor_copy(ob[:], op_ps[mi][:])
                    nc.sync.dma_start(
                        out_d.ap()[c0 + m * 128: c0 + (m + 1) * 128,
                                   n * 512:(n + 1) * 512], ob[:])
            return op_ps

        def outproj_block(j, oT, rzs, skip_norm=(), ns=(0, 1, 2, 3)):
            for h in range(NH):
                if h not in skip_norm:
                    norm_head(j, h, oT[h], rzs[h])
            for n in ns:
                for mp in range(2):
                    op_group(j, n, mp, oT, ps_acc, "acc", list(range(NH)),
                             True, True, True)

        # ---- software pipeline ----
        # Block 0 QKV: all 4 q-head accumulations interleaved per
        # d-chunk so the PE consumes each x/wq chunk-pair as it lands.
        cos0 = cos_t[0][:]
        sin0 = sin_t[0][:]
        q_ps0 = [ps_acc.tile([128, QB], F32, tag="acc", name=f"q_ps0_{h}")
                 for h in range(3)]
        q_ps0.append(ps_att.tile([128, QB], F32, tag="att", name="q_ps0_3"))
        for d in range(NDC):
            for h in range(NH):
                nc.tensor.matmul(
                    q_ps0[h][:],
                    wq_t[d][:, h * 128:(h + 1) * 128],
                    x0_t[d][:], start=(d == 0), stop=(d == NDC - 1))
        qT_cur = []
        for h in range(NH):
            qt = qpool.tile([128, QB], BF16, tag="qT", name=f"qt0_{h}")
            rope(qt, q_ps0[h], cos0, sin0)
            qT_cur.append(qt)
        for g in range(NKV):
            k_group(0, g, x0_t, cos0, sin0)
        for m in range(4):
            v_group(0, m, x0_t)

        LAST = NB - 1
        oT3, rz3 = [], []
        for j in range(NB - 1):
            nkc = 4 * (j + 1)
            xtsn = xts(j + 1)
            cosn = cos_t[j + 1][:]
            sinn = sin_t[j + 1][:]
            oT_cur = [attn_head(j, nkc, qT_cur, 0),
                      attn_head(j, nkc, qT_cur, 1)]
            rz_cur = [z_fin(j, 0, oT_cur[0][1])]
            qT_next = [q_group(j + 1, 0, xtsn, cosn, sinn)]
            oT_cur.append(attn_head(j, nkc, qT_cur, 2))
            rz_cur.append(z_fin(j, 1, oT_cur[1][1]))
            qT_next.append(q_group(j + 1, 1, xtsn, cosn, sinn))
            oT_cur.append(attn_head(j, nkc, qT_cur, 3))
            rz_cur.append(z_fin(j, 2, oT_cur[2][1]))
            qT_next.append(q_group(j + 1, 2, xtsn, cosn, sinn))
            qT_next.append(q_group(j + 1, 3, xtsn, cosn, sinn))
            rz_cur.append(z_fin(j, 3, oT_cur[3][1]))
            for g in range(NKV):
                k_group(j + 1, g, xtsn, cosn, sinn)
            for m in range(4):
                v_group(j + 1, m, xtsn)
            qT_cur = qT_next
            oT_sb = [o for o, _ in oT_cur]
            if j == NB - 2:
                # interleave the last block's heads with outproj(j) halves
                # so the exp-bound chains overlap dense matmul work
                oT3.append(attn_head(LAST, 4 * NB, qT_cur, 0))
                outproj_block(j, oT_sb, rz_cur, ns=(0, 1))
                rz3.append(z_fin(LAST, 0, oT3[0][1]))
                oT3.append(attn_head(LAST, 4 * NB, qT_cur, 1))
                outproj_block(j, oT_sb, rz_cur,
                              skip_norm=(0, 1, 2, 3), ns=(2, 3))
                rz3.append(z_fin(LAST, 1, oT3[1][1]))
                oT3.append(attn_head(LAST, 4 * NB, qT_cur, 2))
                rz3.append(z_fin(LAST, 2, oT3[2][1]))
            else:
                outproj_block(j, oT_sb, rz_cur)
        for h in range(3):
            norm_head(LAST, h, oT3[h][0], rz3[h])
        oT3.append(attn_head(LAST, 4 * NB, qT_cur, 3))
        rz3.append(z_fin(LAST, 3, oT3[3][1]))

        # ---- final outproj: issue h0-2 matmuls for two column groups
        # BEFORE head 3's normalization so the PE queue has ~2.6us of
        # dense work covering the last z chain ----
        oT_f = [o for o, _ in oT3]
        gA = op_group(LAST, 0, 0, oT_f, ps_acc, "acc", [0, 1, 2],
                      True, False, False)
        gB = op_group(LAST, 0, 1, oT_f, ps_att, "att", [0, 1, 2],
                      True, False, False)
        norm_head(LAST, 3, oT_f[3], rz3[3])
        op_cont(LAST, 0, 0, oT_f, gA, [3], False, True, True)
        op_cont(LAST, 0, 1, oT_f, gB, [3], False, True, True)
        for n in (1, 2, 3):
            for mp in range(2):
                op_group(LAST, n, mp, oT_f, ps_acc, "acc", list(range(NH)),
                         True, True, True)

    nc.compile()
    return nc


_NC_CACHE = None


def _get_nc():
    global _NC_CACHE
    if _NC_CACHE is None:
        _NC_CACHE = _build_nc()
    return _NC_CACHE


def _host_prep(inputs):
    """Build the 8 per-core input maps from the full problem inputs."""
    hs = np.asarray(inputs["hidden_state"], dtype=np.float32)
    cos = np.asarray(inputs["freq_cos"], dtype=np.float32)[0, :, 0, :]  # [S,64]
    sin = np.asarray(inputs["freq_sin"], dtype=np.float32)[0, :, 0, :]
    wq = np.asarray(inputs["wq"], dtype=np.float32)
    wk = np.asarray(inputs["wk"], dtype=np.float32)
    wv = np.asarray(inputs["wv"], dtype=np.float32)
    wo = np.asarray(inputs["wo"], dtype=np.float32)

    perm = np.concatenate([np.arange(0, HD, 2), np.arange(1, HD, 2)])  # [128]

    cos2 = np.empty((HD, SEQ), dtype=np.float32)
    sins = np.empty((HD, SEQ), dtype=np.float32)
    cos2[:HALF] = cos.T
    cos2[HALF:] = cos.T
    sins[:HALF] = -sin.T
    sins[HALF:] = sin.T
    cos2 = cos2.astype(ml_dtypes.bfloat16)
    sins = sins.astype(ml_dtypes.bfloat16)

    ki = np.arange(KC)
    # additive causal mask for diagonal chunks: key k > query c -> -1e9
    maskadd = np.where(ki[:, None] > ki[None, :], -1e9,
                       0.0).astype(ml_dtypes.bfloat16)
    ident = np.eye(KC, dtype=ml_dtypes.bfloat16)
    ones128 = np.ones((128, 1), dtype=np.float16)
    ones1 = np.ones((1, 128), dtype=np.float16)

    def tile_pdc(w):
        # [2048, C] -> [128, 16*C]: row p holds chunks d=0..15 contiguously
        c = w.shape[1]
        return np.ascontiguousarray(
            w.reshape(NDC, 128, c).transpose(1, 0, 2).reshape(128, NDC * c))

    # x^T -> [p][j][d][c] so block-j 4-chunk groups are 4KB-contiguous
    xTs = []
    for b in range(BS):
        xT = hs[b].T.astype(ml_dtypes.bfloat16)          # [dim, seq]
        x4 = xT.reshape(NDC, 128, NB, QB).transpose(1, 2, 0, 3)
        xTs.append(np.ascontiguousarray(x4.reshape(128, NB * NDC * QB)))

    in_maps = []
    for c in range(NCORES):
        b, r = divmod(c, TP)
        qcols = np.concatenate(
            [(4 * r + h) * HD + perm for h in range(NH)])
        kcols = np.concatenate(
            [(NKV * r + g) * HD + perm for g in range(NKV)])
        vcols = np.concatenate(
            [(NKV * r + g) * HD + np.arange(HD) for g in range(NKV)])
        worows = np.concatenate(
            [(4 * r + h) * HD + np.arange(HD) for h in range(NH)])
        in_maps.append({
            "xt": xTs[b],
            "wq": tile_pdc(wq[:, qcols].astype(ml_dtypes.bfloat16)),
            "wk": tile_pdc(wk[:, kcols].astype(ml_dtypes.bfloat16)),
            "wv": tile_pdc(wv[:, vcols].astype(ml_dtypes.bfloat16)),
            "wo": np.ascontiguousarray(wo[worows, :]).astype(ml_dtypes.bfloat16),
            "cos2": cos2,
            "sins": sins,
            "maskadd": maskadd,
            "ident": ident,
            "ones128": ones128,
            "ones1": ones1,
        })
    return in_maps


def _run(inputs, trace=False, **trace_kwargs):
    nc = _get_nc()
    in_maps = _host_prep(inputs)
    res = run_bass_kernel_spmd(nc, in_maps, list(range(NCORES)),
                               trace=trace, **trace_kwargs)
    out = np.zeros((BS, SEQ, DIM), dtype=np.float32)
    for c in range(NCORES):
        out[c // TP] += np.asarray(res.results[c]["out"], dtype=np.float32)
    return out, res


def kernel(**inputs) -> np.ndarray:
    out, _ = _run(inputs, trace=False)
    return out


# revision 15
# speedup vs baseline: 1.2923x; 1.0386x over previous
"""GQA attention (bs=2, seq=2048, dim=2048, 16 q-heads / 8 kv-heads, hd=128)
on 8 Trainium2 NeuronCores.

Sharding: 2-way data parallel (batch) x 4-way tensor parallel (heads, kv
groups intact).  Core c handles batch c//4 and q-heads [4*(c%4), 4*(c%4)+4)
(kv-heads [2*(c%4), 2*(c%4)+2)).  Each core computes a partial output
projection (row-split wo); the all-reduce over the 4 TP ranks is done on the
host while gathering (bf16 partials summed in f32).

Device kernel (per core):
  - all inputs bf16 (weights, x^T) -> FWL-eligible stationaries, half DMA.
  - host supplies X^T (so `dim` lands on partitions for every projection)
    and rotate-half permuted wq/wk, so RoPE is 4 DVE ops per tile.
  - scores are computed transposed (P^T[k, q]) which makes PV and the
    output projection transpose-free.
  - causal masking is additive: a [128,128] -1e9 strictly-lower matrix is
    accumulated into the scores PSUM bank by a tiny N=128 matmul
    (identity stationary) before the score matmul, so exp() produces
    exact zeros and the DVE mask multiply disappears from the
    exp->PV chain.
  - softmax row-sums: P^T chunks are accumulated into a [128, QB] fp16
    SBUF tile by DVE adds; one all-ones [128,1] matmul per head-block
    reduces over partitions; 1/z = exp(-ln(z)) on the scalar engine
    (Ln and Exp share one ACT table set), avoiding any DMA round-trip;
    a [1,128] ones matmul broadcasts 1/z back to 128 partitions for the
    DVE normalization multiply.

Perf notes (vs the first working version, 312.4us -> target ~270us):
  - 96 warmup matmuls (>3.4us busy) so the PE HAM clock-gate opens at
    ~3.4us instead of 50us; previously the whole DMA-fed ramp ran at
    1.2GHz.
  - startup DMAs spread over 4 engine queues (scalar/vector for wq,
    sync/gpsimd for x block 0) and block-0 Q accumulates all 4 heads
    per d-chunk, so the PE consumes each 2x128KB chunk-pair (863ns) at
    the pace DMA delivers it.
  - x/wq/wk/wv are host-retiled so every DMA line is 1-4KB contiguous;
    blocks 1-3 of x load as flat [128, 2048] tiles (4KB lines); x stays
    SBUF-resident all kernel (~64KB/partition).
  - attention PV matmuls issue two chunks behind the score matmuls so
    the in-order PE queue never waits on the ACT exp chain.
  - RoPE first evacuates PSUM via one ACT copy (bf16), freeing the
    accumulation bank in ~0.6us instead of ~2us and running the 4 DVE
    ops in 2x packed mode.
  - the final outproj issues h0-2 matmuls for two column groups before
    head 3's normalization so the last z chain is hidden.
"""

from contextlib import ExitStack

import ml_dtypes
import numpy as np

import concourse.bass as bass
import concourse.tile as tile
from concourse import bacc, mybir
from concourse.bass_utils import run_bass_kernel_spmd

F32 = mybir.dt.float32
BF16 = mybir.dt.bfloat16
F16 = mybir.dt.float16

BS = 2
SEQ = 2048
DIM = 2048
N_HEADS = 16
N_KV_HEADS = 8
HD = 128
HALF = HD // 2

NCORES = 8
TP = 4                     # tensor-parallel ranks per batch
NH = N_HEADS // TP         # q heads per core = 4
NKV = N_KV_HEADS // TP     # kv heads per core = 2
QB = 512                   # q block (free dim of score matmuls)
KC = 128                   # k chunk (partition dim of P^T tiles)
DC = 128                   # contraction chunk (partitions)
NDC = DIM // DC            # 16
NB = SEQ // QB             # 4 seq blocks
SCALE = 1.0 / np.sqrt(HD)


def _build_nc():
    nc = bacc.Bacc("TRN2", target_bir_lowering=False, debug=False,
                   num_devices=NCORES)
    # host-retiled layouts: [partition][...contiguous cols...]
    xt_d = nc.declare_dram_parameter("xt", [128, NB * NDC * QB], BF16,
                                     isOutput=False)   # [p][j][d][c]
    wq_d = nc.declare_dram_parameter("wq", [128, NDC * NH * HD], BF16,
                                     isOutput=False)   # [p][d][h*128+c]
    wk_d = nc.declare_dram_parameter("wk", [128, NDC * NKV * HD], BF16,
                                     isOutput=False)
    wv_d = nc.declare_dram_parameter("wv", [128, NDC * NKV * HD], BF16,
                                     isOutput=False)
    wo_d = nc.declare_dram_parameter("wo", [NH * HD, DIM], BF16,
                                     isOutput=False)
    cos_d = nc.declare_dram_parameter("cos2", [HD, SEQ], BF16, isOutput=False)
    sin_d = nc.declare_dram_parameter("sins", [HD, SEQ], BF16, isOutput=False)
    msk_d = nc.declare_dram_parameter("maskadd", [KC, KC], BF16,
                                      isOutput=False)
    idn_d = nc.declare_dram_parameter("ident", [KC, KC], BF16, isOutput=False)
    on128_d = nc.declare_dram_parameter("ones128", [128, 1], F16,
                                        isOutput=False)
    on1_d = nc.declare_dram_parameter("ones1", [1, 128], F16, isOutput=False)
    out_d = nc.declare_dram_parameter("out", [SEQ, DIM], BF16, isOutput=True)

    with tile.TileContext(nc) as tc, ExitStack() as ctx:
        wpool = ctx.enter_context(tc.tile_pool(name="weights", bufs=1))
        kvpool = ctx.enter_context(tc.tile_pool(name="kv", bufs=1))
        xpool = ctx.enter_context(tc.tile_pool(name="xt", bufs=1))
        qpool = ctx.enter_context(tc.tile_pool(name="qT", bufs=8))
        ppool = ctx.enter_context(tc.tile_pool(name="pT", bufs=8))
        ospool = ctx.enter_context(tc.tile_pool(name="osb", bufs=8))
        zpool = ctx.enter_context(tc.tile_pool(name="zacc", bufs=3))
        npool = ctx.enter_context(tc.tile_pool(name="norm", bufs=1))
        tpool = ctx.enter_context(tc.tile_pool(name="tmp", bufs=2))
        obpool = ctx.enter_context(tc.tile_pool(name="outb", bufs=8))
        ps_acc = ctx.enter_context(tc.tile_pool(name="ps_acc", bufs=3,
                                                space="PSUM"))
        ps_sc = ctx.enter_context(tc.tile_pool(name="ps_sc", bufs=3,
                                               space="PSUM"))
        ps_att = ctx.enter_context(tc.tile_pool(name="ps_att", bufs=2,
                                                space="PSUM"))

        # ---- persistent weights/constants in SBUF ----
        # wq in 4 group tiles (one 512KB 4KB-line DMA each)
        wq_g = [wpool.tile([128, 4 * NH * HD], BF16, tag=f"wq{g}",
                           name=f"wq{g}") for g in range(4)]
        wq_t = [wq_g[d // 4][:, (d % 4) * 512:(d % 4 + 1) * 512]
                for d in range(NDC)]
        wk_sb = wpool.tile([128, NDC * NKV * HD], BF16, tag="wk", name="wk_sb")
        wv_sb = wpool.tile([128, NDC * NKV * HD], BF16, tag="wv", name="wv_sb")
        wk_t = [wk_sb[:, d * NKV * HD:(d + 1) * NKV * HD] for d in range(NDC)]
        wv_t = [wv_sb[:, d * NKV * HD:(d + 1) * NKV * HD] for d in range(NDC)]
        wo_sb = wpool.tile([128, NH * 4 * 512], BF16, tag="wo", name="wo_sb")
        cos_sb = wpool.tile([128, SEQ], BF16, tag="cos", name="cos_sb")
        sin_sb = wpool.tile([128, SEQ], BF16, tag="sin", name="sin_sb")
        cos_t = [cos_sb[:, j * QB:(j + 1) * QB] for j in range(NB)]
        sin_t = [sin_sb[:, j * QB:(j + 1) * QB] for j in range(NB)]

        # x resident for the whole kernel, one tile per 4-chunk group
        # (512KB DMAs with 4KB contiguous lines)
        xg_t = {(j, g): xpool.tile([128, 4 * QB], BF16, tag=f"x{j}_{g}",
                                   name=f"x{j}_{g}")
                for j in range(NB) for g in range(4)}

        def xts(j):
            return [xg_t[j, d // 4][:, (d % 4) * QB:(d % 4 + 1) * QB]
                    for d in range(NDC)]
        x0_t = xts(0)

        # ---- HAM warmup: >3.4us of dummy matmuls flips the PE clock
        # gate to 8/8 before the first real matmuls arrive ----
        warm_w = wpool.tile([128, 128], BF16, tag="warmw", name="warmw")
        warm_x = wpool.tile([128, 64], BF16, tag="warmx", name="warmx")
        nc.vector.memset(warm_w[:], 0.0)
        nc.vector.memset(warm_x[:], 0.0)
        warm_ps = ps_sc.tile([128, 64], F32, tag="sc", name="warm_ps")
        for _ in range(96):
            nc.tensor.matmul(warm_ps[:], warm_w[:], warm_x[:],
                             start=True, stop=True)

        # ---- startup DMAs: the ramp is aggregate-HBM-bound (~310GB/s
        # across all queues), so transfers are issued in strict
        # need-order round-robin over the 3 DMA-capable queues, with
        # x2/x3 (needed at ~85/130us) strictly after all critical
        # bytes.  Q-phase wq/x0 groups are split into partition halves
        # across two queues so each group lands ~2x sooner. ----
        import itertools
        qcycle = itertools.cycle([nc.scalar, nc.sync, nc.gpsimd])

        def rr_dma(dst, src):
            next(qcycle).dma_start(dst, src)

        def rr_dma_split(dst, src):
            next(qcycle).dma_start(dst[0:64, :], src[0:64, :])
            next(qcycle).dma_start(dst[64:128, :], src[64:128, :])

        def xg_src(j, g):
            c0 = (j * NDC + 4 * g) * QB
            return xt_d.ap()[:, c0:c0 + 4 * QB]

        for g in range(4):
            rr_dma_split(wq_g[g][:], wq_d.ap()[:, g * 2048:(g + 1) * 2048])
            rr_dma_split(xg_t[0, g][:], xg_src(0, g))
        rr_dma(cos_sb[:], cos_d.ap()[:])
        rr_dma(sin_sb[:], sin_d.ap()[:])
        rr_dma(wk_sb[:, 0:2048], wk_d.ap()[:, 0:2048])
        rr_dma(wk_sb[:, 2048:4096], wk_d.ap()[:, 2048:4096])
        ident = wpool.tile([KC, KC], BF16, tag="ident", name="ident")
        maskA = wpool.tile([KC, KC], BF16, tag="maskA", name="maskA")
        ones128 = wpool.tile([128, 1], F16, tag="ones128", name="ones128")
        ones1 = wpool.tile([1, 128], F16, tag="ones1", name="ones1")
        cst_eng = next(qcycle)
        cst_eng.dma_start(ident[:], idn_d.ap()[:])
        cst_eng.dma_start(maskA[:], msk_d.ap()[:])
        cst_eng.dma_start(ones128[:], on128_d.ap()[:])
        cst_eng.dma_start(ones1[:], on1_d.ap()[:])
        rr_dma(wv_sb[:, 0:2048], wv_d.ap()[:, 0:2048])
        rr_dma(wv_sb[:, 2048:4096], wv_d.ap()[:, 2048:4096])
        for g in range(4):
            rr_dma(xg_t[1, g][:], xg_src(1, g))
        # wo resident load: [512, 2048] -> [128, (h n c)], 4KB lines
        for h in range(NH):
            rr_dma(wo_sb[:, h * 2048:(h + 1) * 2048],
                   wo_d.ap()[h * 128:(h + 1) * 128, :])
        for g in range(4):
            rr_dma(xg_t[2, g][:], xg_src(2, g))
        for g in range(4):
            rr_dma(xg_t[3, g][:], xg_src(3, g))

        # ---- persistent K^T / V for the whole sequence ----
        kT = [kvpool.tile([128, SEQ], BF16, tag=f"kT{g}", name=f"kT{g}")
              for g in range(NKV)]
        # v_sb columns: [kchunk c][kv head g] -> [:, c*256 + g*128 :+128]
        v_sb = kvpool.tile([128, (SEQ // KC) * NKV * HD], F16, tag="v", name="v_sb")
        assert v_sb.shape[1] == 4096

        def rope(dst, src_ps, cos_t, sin_t):
            """dst = src*cos2 + swap_halves(src)*sins  (dst bf16 SBUF).

            The half-swap muls must read PSUM (DVE only allows a
            partition-base shift when one operand is PSUM); the ACT
            copy in parallel frees the PSUM bank, and the remaining
            cos-mul + add run all-SBUF bf16 in 2x packed mode."""
            nc.vector.tensor_mul(dst[0:64, :], src_ps[64:128, :],
                                 sin_t[0:64, :])
            nc.vector.tensor_mul(dst[64:128, :], src_ps[0:64, :],
                                 sin_t[64:128, :])
            t0 = tpool.tile([128, QB], BF16, tag="ropesrc", name="ropesrc")
            nc.scalar.copy(t0[:], src_ps[:])
            tmp = tpool.tile([128, QB], BF16, tag="ropetmp", name="ropetmp")
            nc.vector.tensor_mul(tmp[:], t0[:], cos_t[:])
            nc.vector.tensor_add(dst[:], dst[:], tmp[:])

        def q_group(j, h, xts_, cos_t, sin_t):
            q_ps = ps_acc.tile([128, QB], F32, tag="acc", name=f"q_ps{j}_{h}")
            for d in range(NDC):
                nc.tensor.matmul(
                    q_ps[:],
                    wq_t[d][:, h * 128:(h + 1) * 128],
                    xts_[d][:], start=(d == 0), stop=(d == NDC - 1))
            qt = qpool.tile([128, QB], BF16, tag="qT", name=f"qt{j}_{h}")
            rope(qt, q_ps, cos_t, sin_t)
            return qt

        def k_group(j, g, xts_, cos_t, sin_t):
            c0 = j * QB
            k_ps = ps_acc.tile([128, QB], F32, tag="acc", name=f"k_ps{j}_{g}")
            for d in range(NDC):
                nc.tensor.matmul(
                    k_ps[:],
                    wk_t[d][:, g * 128:(g + 1) * 128],
                    xts_[d][:], start=(d == 0), stop=(d == NDC - 1))
            rope(kT[g][:, c0:c0 + QB], k_ps, cos_t, sin_t)

        def v_group(j, m, xts_):
            v_ps = ps_acc.tile([128, NKV * HD], F32, tag="acc",
                               name=f"v_ps{j}_{m}")
            for d in range(NDC):
                nc.tensor.matmul(
                    v_ps[:],
                    xts_[d][:, m * 128:(m + 1) * 128],
                    wv_t[d][:],
                    start=(d == 0), stop=(d == NDC - 1))
            kc = 4 * j + m
            with nc.allow_low_precision(reason="V in fp16 (11-bit) is plenty"):
                nc.scalar.copy(v_sb[:, kc * 256:(kc + 1) * 256], v_ps[:])

        def wo_ap(n, h):
            return wo_sb[:, h * 2048 + n * 512: h * 2048 + (n + 1) * 512]

        def attn_head(j, nkc, qT, h):
            """Scores/exp/PV for one head; PV issues two chunks behind
            the score matmuls so the in-order PE queue never waits on
            the ACT exp chain."""
            g = h // 2
            o_ps = ps_att.tile([128, QB], F32, tag="att", name=f"o_ps{j}_{h}")
            zacc = zpool.tile([128, QB], F16, tag="zacc",
                              name=f"zacc{j}_{h}")
            pts = [None] * nkc
            offs = [max(0, (kc - 4 * j) * 128) for kc in range(nkc)]

            def issue_score(kc):
                off = offs[kc]
                sc_ps = ps_sc.tile([128, QB], F32, tag="sc",
                                   name=f"sc{j}_{h}_{kc}")
                if kc >= 4 * j:
                    # additive causal mask: -1e9 above the diagonal, via
                    # a tiny identity-stationary matmul into the bank
                    nc.tensor.matmul(sc_ps[:, off:off + KC], ident[:],
                                     maskA[:], start=True, stop=False)
                    nc.tensor.matmul(sc_ps[:, off:QB],
                                     kT[g][:, kc * 128:(kc + 1) * 128],
                                     qT[h][:, off:QB], start=False, stop=True)
                else:
                    nc.tensor.matmul(sc_ps[:, off:QB],
                                     kT[g][:, kc * 128:(kc + 1) * 128],
                                     qT[h][:, off:QB], start=True, stop=True)
                pt = ppool.tile([128, QB], F16, tag="pT",
                                name=f"pt{j}_{h}_{kc}")
                nc.scalar.activation(pt[:, off:QB], sc_ps[:, off:QB],
                                     mybir.ActivationFunctionType.Exp,
                                     scale=float(SCALE))
                with nc.allow_low_precision(
                        reason="softmax z accum in fp16 (11-bit) is plenty"):
                    if kc == 0:
                        nc.vector.tensor_copy(zacc[:], pt[:])
                    else:
                        nc.vector.tensor_add(zacc[:, off:QB],
                                             zacc[:, off:QB],
                                             pt[:, off:QB])
                pts[kc] = pt

            def issue_pv(kc):
                off = offs[kc]
                nc.tensor.matmul(o_ps[:, off:QB],
                                 v_sb[:, kc * 256 + g * 128:
                                      kc * 256 + (g + 1) * 128],
                                 pts[kc][:, off:QB], start=(kc == 0),
                                 stop=(kc == nkc - 1))

            issue_score(0)
            issue_score(1)
            for kc in range(2, nkc):
                issue_score(kc)
                issue_pv(kc - 2)
            issue_pv(nkc - 2)
            issue_pv(nkc - 1)
            # stage unnormalized O' now (frees the PSUM bank quickly); the
            # z finalize is issued separately (z_fin) one head later
            o_sb = ospool.tile([128, QB], BF16, tag="osb", name=f"o_sb{j}_{h}")
            nc.scalar.copy(o_sb[:], o_ps[:])
            return (o_sb, zacc)

        def z_fin(j, h, zacc):
            # z row = ones^T @ zacc (partition reduce), then reshape the z
            # row to [128,4] so the reciprocal runs on all 128 DVE lanes;
            # reshape hops ride the GPSIMD queue, which is idle once the
            # x loads finish (sync carries output writes and would add
            # multi-us queueing latency at the tail)
            z_ps = ps_sc.tile([1, QB], F32, tag="sc", name=f"z_ps{j}_{h}")
            nc.tensor.matmul(z_ps[:], ones128[:], zacc[:],
                             start=True, stop=True)
            z_sb = npool.tile([1, QB], F32, tag="z", bufs=4,
                              name=f"z_sb{j}_{h}")
            nc.scalar.copy(z_sb[:], z_ps[:])
            zc = npool.tile([128, QB // 128], F32, tag="zc", bufs=4,
                            name=f"zc{j}_{h}")
            nc.gpsimd.dma_start(zc[:], z_sb[:])
            rzc = npool.tile([128, QB // 128], F16, tag="rzc", bufs=4,
                             name=f"rzc{j}_{h}")
            with nc.allow_low_precision(
                    reason="1/z in fp16 (11-bit mantissa) is plenty"):
                nc.vector.reciprocal(rzc[:], zc[:])
            rz = npool.tile([1, QB], F16, tag="rz", bufs=6,
                            name=f"rz{j}_{h}")
            nc.gpsimd.dma_start(rz[:], rzc[:])
            return rz

        def norm_head(j, h, o_sb, rz):
            # o_sb *= broadcast(1/z) (in place)
            zb_ps = ps_sc.tile([128, QB], F32, tag="sc", name=f"zb{j}_{h}")
            nc.tensor.matmul(zb_ps[:], ones1[:], rz[:], start=True, stop=True)
            nc.vector.tensor_mul(o_sb[:], o_sb[:], zb_ps[:])

        def op_group(j, n, mp, oT, pool, tag, heads, start, stop, evac):
            """Issue outproj matmuls for heads `heads` of column group
            (n, mp) into 2 PSUM banks from `pool`; returns the banks."""
            op_ps = [pool.tile([128, 512], F32, tag=tag,
                               name=f"op{j}_{n}_{mp}_{m}")
                     for m in range(2)]
            return op_cont(j, n, mp, oT, op_ps, heads, start, stop, evac)

        def op_cont(j, n, mp, oT, op_ps, heads, start, stop, evac):
            c0 = j * QB
            for h in heads:
                for mi in range(2):
                    m = 2 * mp + mi
                    nc.tensor.matmul(
                        op_ps[mi][:],
                        oT[h][:, m * 128:(m + 1) * 128],
                        wo_ap(n, h),
                        start=(h == heads[0] and start),
                        stop=(h == heads[-1] and stop))
            if evac:
                for mi in range(2):
                    m = 2 * mp + mi
                    ob = obpool.tile([128, 512], BF16, tag="ob",
                                     name=f"ob{j}_{n}_{m}")
                    # split PSUM->SBUF evacuations between ACT and DVE
                    if mi == 0:
                        nc.scalar.copy(ob[:], op_ps[mi][:])
                    else:
                        nc.vector.tensor_copy(ob[:], op_ps[mi][:])
                    oeng = nc.sync if (n + mi) % 2 == 0 else nc.gpsimd
                    oeng.dma_start(
                        out_d.ap()[c0 + m * 128: c0 + (m + 1) * 128,
                                   n * 512:(n + 1) * 512], ob[:])
            return op_ps

        def outproj_block(j, oT, rzs, skip_norm=(), ns=(0, 1, 2, 3)):
            for h in range(NH):
                if h not in skip_norm:
                    norm_head(j, h, oT[h], rzs[h])
            for n in ns:
                for mp in range(2):
                    op_group(j, n, mp, oT, ps_acc, "acc", list(range(NH)),
                             True, True, True)

        # ---- software pipeline ----
        # Block 0 QKV: all 4 q-head accumulations interleaved per
        # d-chunk so the PE consumes each x/wq chunk-pair as it lands.
        cos0 = cos_t[0]
        sin0 = sin_t[0]
        q_ps0 = [ps_acc.tile([128, QB], F32, tag="acc", name=f"q_ps0_{h}")
                 for h in range(3)]
        q_ps0.append(ps_att.tile([128, QB], F32, tag="att", name="q_ps0_3"))
        for d in range(NDC):
            for h in range(NH):
                nc.tensor.matmul(
                    q_ps0[h][:],
                    wq_t[d][:, h * 128:(h + 1) * 128],
                    x0_t[d][:], start=(d == 0), stop=(d == NDC - 1))
        qT_cur = []
        for h in range(NH):
            qt = qpool.tile([128, QB], BF16, tag="qT", name=f"qt0_{h}")
            rope(qt, q_ps0[h], cos0, sin0)
            qT_cur.append(qt)
        for g in range(NKV):
            k_group(0, g, x0_t, cos0, sin0)
        for m in range(4):
            v_group(0, m, x0_t)

        LAST = NB - 1
        oT3, rz3 = [], []
        for j in range(NB - 1):
            nkc = 4 * (j + 1)
            xtsn = xts(j + 1)
            cosn = cos_t[j + 1]
            sinn = sin_t[j + 1]
            oT_cur = [attn_head(j, nkc, qT_cur, 0),
                      attn_head(j, nkc, qT_cur, 1)]
            rz_cur = [z_fin(j, 0, oT_cur[0][1])]
            qT_next = [q_group(j + 1, 0, xtsn, cosn, sinn)]
            oT_cur.append(attn_head(j, nkc, qT_cur, 2))
            rz_cur.append(z_fin(j, 1, oT_cur[1][1]))
            qT_next.append(q_group(j + 1, 1, xtsn, cosn, sinn))
            oT_cur.append(attn_head(j, nkc, qT_cur, 3))
            rz_cur.append(z_fin(j, 2, oT_cur[2][1]))
            qT_next.append(q_group(j + 1, 2, xtsn, cosn, sinn))
            qT_next.append(q_group(j + 1, 3, xtsn, cosn, sinn))
            rz_cur.append(z_fin(j, 3, oT_cur[3][1]))
            for g in range(NKV):
                k_group(j + 1, g, xtsn, cosn, sinn)
            for m in range(4):
                v_group(j + 1, m, xtsn)
            qT_cur = qT_next
            oT_sb = [o for o, _ in oT_cur]
            if j == NB - 2:
                # ALL of the last block's attention + z chains run before
                # outproj(j)'s second half, so the 32 matmuls of
                # ns=(2,3) (~7us) cover the final z DMA round-trip and
                # outproj(LAST) starts unblocked
                oT3.append(attn_head(LAST, 4 * NB, qT_cur, 0))
                outproj_block(j, oT_sb, rz_cur, ns=(0, 1))
                rz3.append(z_fin(LAST, 0, oT3[0][1]))
                oT3.append(attn_head(LAST, 4 * NB, qT_cur, 1))
                rz3.append(z_fin(LAST, 1, oT3[1][1]))
                oT3.append(attn_head(LAST, 4 * NB, qT_cur, 2))
                rz3.append(z_fin(LAST, 2, oT3[2][1]))
                oT3.append(attn_head(LAST, 4 * NB, qT_cur, 3))
                rz3.append(z_fin(LAST, 3, oT3[3][1]))
                outproj_block(j, oT_sb, rz_cur,
                              skip_norm=(0, 1, 2, 3), ns=(2, 3))
            else:
                outproj_block(j, oT_sb, rz_cur)
        outproj_block(LAST, [o for o, _ in oT3], rz3)

    nc.compile()
    return nc


_NC_CACHE = None


def _get_nc():
    global _NC_CACHE
    if _NC_CACHE is None:
        _NC_CACHE = _build_nc()
    return _NC_CACHE


def _host_prep(inputs):
    """Build the 8 per-core input maps from the full problem inputs."""
    hs = np.asarray(inputs["hidden_state"], dtype=np.float32)
    cos = np.asarray(inputs["freq_cos"], dtype=np.float32)[0, :, 0, :]  # [S,64]
    sin = np.asarray(inputs["freq_sin"], dtype=np.float32)[0, :, 0, :]
    wq = np.asarray(inputs["wq"], dtype=np.float32)
    wk = np.asarray(inputs["wk"], dtype=np.float32)
    wv = np.asarray(inputs["wv"], dtype=np.float32)
    wo = np.asarray(inputs["wo"], dtype=np.float32)

    perm = np.concatenate([np.arange(0, HD, 2), np.arange(1, HD, 2)])  # [128]

    cos2 = np.empty((HD, SEQ), dtype=np.float32)
    sins = np.empty((HD, SEQ), dtype=np.float32)
    cos2[:HALF] = cos.T
    cos2[HALF:] = cos.T
    sins[:HALF] = -sin.T
    sins[HALF:] = sin.T
    cos2 = cos2.astype(ml_dtypes.bfloat16)
    sins = sins.astype(ml_dtypes.bfloat16)

    ki = np.arange(KC)
    # additive causal mask for diagonal chunks: key k > query c -> -1e9
    maskadd = np.where(ki[:, None] > ki[None, :], -1e9,
                       0.0).astype(ml_dtypes.bfloat16)
    ident = np.eye(KC, dtype=ml_dtypes.bfloat16)
    ones128 = np.ones((128, 1), dtype=np.float16)
    ones1 = np.ones((1, 128), dtype=np.float16)

    def tile_pdc(w):
        # [2048, C] -> [128, 16*C]: row p holds chunks d=0..15 contiguously
        c = w.shape[1]
        return np.ascontiguousarray(
            w.reshape(NDC, 128, c).transpose(1, 0, 2).reshape(128, NDC * c))

    # x^T -> [p][j][d][c] so block-j 4-chunk groups are 4KB-contiguous
    xTs = []
    for b in range(BS):
        xT = hs[b].T.astype(ml_dtypes.bfloat16)          # [dim, seq]
        x4 = xT.reshape(NDC, 128, NB, QB).transpose(1, 2, 0, 3)
        xTs.append(np.ascontiguousarray(x4.reshape(128, NB * NDC * QB)))

    in_maps = []
    for c in range(NCORES):
        b, r = divmod(c, TP)
        qcols = np.concatenate(
            [(4 * r + h) * HD + perm for h in range(NH)])
        kcols = np.concatenate(
            [(NKV * r + g) * HD + perm for g in range(NKV)])
        vcols = np.concatenate(
            [(NKV * r + g) * HD + np.arange(HD) for g in range(NKV)])
        worows = np.concatenate(
            [(4 * r + h) * HD + np.arange(HD) for h in range(NH)])
        in_maps.append({
            "xt": xTs[b],
            "wq": tile_pdc(wq[:, qcols].astype(ml_dtypes.bfloat16)),
            "wk": tile_pdc(wk[:, kcols].astype(ml_dtypes.bfloat16)),
            "wv": tile_pdc(wv[:, vcols].astype(ml_dtypes.bfloat16)),
            "wo": np.ascontiguousarray(wo[worows, :]).astype(ml_dtypes.bfloat16),
            "cos2": cos2,
            "sins": sins,
            "maskadd": maskadd,
            "ident": ident,
            "ones128": ones128,
            "ones1": ones1,
        })
    return in_maps


def _run(inputs, trace=False, **trace_kwargs):
    nc = _get_nc()
    in_maps = _host_prep(inputs)
    res = run_bass_kernel_spmd(nc, in_maps, list(range(NCORES)),
                               trace=trace, **trace_kwargs)
    out = np.zeros((BS, SEQ, DIM), dtype=np.float32)
    for c in range(NCORES):
        out[c // TP] += np.asarray(res.results[c]["out"], dtype=np.float32)
    return out, res


def kernel(**inputs) -> np.ndarray:
    out, _ = _run(inputs, trace=False)
    return out


# revision 17
# speedup vs baseline: 1.3591x; 1.0517x over previous
"""GQA attention (bs=2, seq=2048, dim=2048, 16 q-heads / 8 kv-heads, hd=128)
on 8 Trainium2 NeuronCores.

Sharding: 2-way data parallel (batch) x 4-way tensor parallel (heads, kv
groups intact).  Core c handles batch c//4 and q-heads [4*(c%4), 4*(c%4)+4)
(kv-heads [2*(c%4), 2*(c%4)+2)).  Each core computes a partial output
projection (row-split wo); the all-reduce over the 4 TP ranks is done on the
host while gathering (bf16 partials summed in f32).

Device kernel (per core):
  - all inputs bf16 (weights, x^T) -> FWL-eligible stationaries, half DMA.
  - host supplies X^T (so `dim` lands on partitions for every projection)
    and rotate-half permuted wq/wk, so RoPE is 4 DVE ops per tile.
  - scores are computed transposed (P^T[k, q]) which makes PV and the
    output projection transpose-free.
  - causal masking is additive: a [128,128] -1e9 strictly-lower matrix is
    accumulated into the scores PSUM bank by a tiny N=128 matmul
    (identity stationary) before the score matmul, so exp() produces
    exact zeros and the DVE mask multiply disappears from the
    exp->PV chain.
  - softmax row-sums: P^T chunks are accumulated into a [128, QB] fp16
    SBUF tile by DVE adds; one all-ones [128,1] matmul per head-block
    reduces over partitions; 1/z = exp(-ln(z)) on the scalar engine
    (Ln and Exp share one ACT table set), avoiding any DMA round-trip;
    a [1,128] ones matmul broadcasts 1/z back to 128 partitions for the
    DVE normalization multiply.

Perf notes (vs the first working version, 312.4us -> target ~270us):
  - 96 warmup matmuls (>3.4us busy) so the PE HAM clock-gate opens at
    ~3.4us instead of 50us; previously the whole DMA-fed ramp ran at
    1.2GHz.
  - startup DMAs spread over 4 engine queues (scalar/vector for wq,
    sync/gpsimd for x block 0) and block-0 Q accumulates all 4 heads
    per d-chunk, so the PE consumes each 2x128KB chunk-pair (863ns) at
    the pace DMA delivers it.
  - x/wq/wk/wv are host-retiled so every DMA line is 1-4KB contiguous;
    blocks 1-3 of x load as flat [128, 2048] tiles (4KB lines); x stays
    SBUF-resident all kernel (~64KB/partition).
  - attention PV matmuls issue two chunks behind the score matmuls so
    the in-order PE queue never waits on the ACT exp chain.
  - RoPE first evacuates PSUM via one ACT copy (bf16), freeing the
    accumulation bank in ~0.6us instead of ~2us and running the 4 DVE
    ops in 2x packed mode.
  - the final outproj issues h0-2 matmuls for two column groups before
    head 3's normalization so the last z chain is hidden.
"""

from contextlib import ExitStack

import ml_dtypes
import numpy as np

import concourse.bass as bass
import concourse.tile as tile
from concourse import bacc, mybir
from concourse.bass_utils import run_bass_kernel_spmd

F32 = mybir.dt.float32
BF16 = mybir.dt.bfloat16
F16 = mybir.dt.float16

BS = 2
SEQ = 2048
DIM = 2048
N_HEADS = 16
N_KV_HEADS = 8
HD = 128
HALF = HD // 2

NCORES = 8
TP = 4                     # tensor-parallel ranks per batch
NH = N_HEADS // TP         # q heads per core = 4
NKV = N_KV_HEADS // TP     # kv heads per core = 2
QB = 512                   # q block (free dim of score matmuls)
KC = 128                   # k chunk (partition dim of P^T tiles)
DC = 128                   # contraction chunk (partitions)
NDC = DIM // DC            # 16
NB = SEQ // QB             # 4 seq blocks
SCALE = 1.0 / np.sqrt(HD)


def _build_nc():
    nc = bacc.Bacc("TRN2", target_bir_lowering=False, debug=False,
                   num_devices=NCORES)
    # host-retiled layouts: [partition][...contiguous cols...]
    xt_d = nc.declare_dram_parameter("xt", [128, NB * NDC * QB], BF16,
                                     isOutput=False)   # [p][j][d][c]
    wq_d = nc.declare_dram_parameter("wq", [128, NDC * NH * HD], BF16,
                                     isOutput=False)   # [p][d][h*128+c]
    wk_d = nc.declare_dram_parameter("wk", [128, NDC * NKV * HD], BF16,
                                     isOutput=False)
    wv_d = nc.declare_dram_parameter("wv", [128, NDC * NKV * HD], BF16,
                                     isOutput=False)
    wo_d = nc.declare_dram_parameter("wo", [NH * HD, DIM], BF16,
                                     isOutput=False)
    cos_d = nc.declare_dram_parameter("cos2", [HD, SEQ], BF16, isOutput=False)
    sin_d = nc.declare_dram_parameter("sins", [HD, SEQ], BF16, isOutput=False)
    msk_d = nc.declare_dram_parameter("maskadd", [KC, KC], BF16,
                                      isOutput=False)
    idn_d = nc.declare_dram_parameter("ident", [KC, KC], BF16, isOutput=False)
    on128_d = nc.declare_dram_parameter("ones128", [128, 1], F16,
                                        isOutput=False)
    on1_d = nc.declare_dram_parameter("ones1", [1, 128], F16, isOutput=False)
    out_d = nc.declare_dram_parameter("out", [SEQ, DIM], BF16, isOutput=True)

    with tile.TileContext(nc) as tc, ExitStack() as ctx:
        wpool = ctx.enter_context(tc.tile_pool(name="weights", bufs=1))
        kvpool = ctx.enter_context(tc.tile_pool(name="kv", bufs=1))
        xpool = ctx.enter_context(tc.tile_pool(name="xt", bufs=1))
        qpool = ctx.enter_context(tc.tile_pool(name="qT", bufs=8))
        ppool = ctx.enter_context(tc.tile_pool(name="pT", bufs=8))
        ospool = ctx.enter_context(tc.tile_pool(name="osb", bufs=8))
        zpool = ctx.enter_context(tc.tile_pool(name="zacc", bufs=3))
        npool = ctx.enter_context(tc.tile_pool(name="norm", bufs=1))
        tpool = ctx.enter_context(tc.tile_pool(name="tmp", bufs=2))
        obpool = ctx.enter_context(tc.tile_pool(name="outb", bufs=8))
        ps_acc = ctx.enter_context(tc.tile_pool(name="ps_acc", bufs=3,
                                                space="PSUM"))
        ps_sc = ctx.enter_context(tc.tile_pool(name="ps_sc", bufs=3,
                                               space="PSUM"))
        ps_att = ctx.enter_context(tc.tile_pool(name="ps_att", bufs=2,
                                                space="PSUM"))

        # ---- persistent weights/constants in SBUF ----
        # wq in 4 group tiles (one 512KB 4KB-line DMA each)
        wq_g = [wpool.tile([128, 4 * NH * HD], BF16, tag=f"wq{g}",
                           name=f"wq{g}") for g in range(4)]
        wq_t = [wq_g[d // 4][:, (d % 4) * 512:(d % 4 + 1) * 512]
                for d in range(NDC)]
        wk_sb = wpool.tile([128, NDC * NKV * HD], BF16, tag="wk", name="wk_sb")
        wv_sb = wpool.tile([128, NDC * NKV * HD], BF16, tag="wv", name="wv_sb")
        wk_t = [wk_sb[:, d * NKV * HD:(d + 1) * NKV * HD] for d in range(NDC)]
        wv_t = [wv_sb[:, d * NKV * HD:(d + 1) * NKV * HD] for d in range(NDC)]
        wo_sb = wpool.tile([128, NH * 4 * 512], BF16, tag="wo", name="wo_sb")
        cos_sb = wpool.tile([128, SEQ], BF16, tag="cos", name="cos_sb")
        sin_sb = wpool.tile([128, SEQ], BF16, tag="sin", name="sin_sb")
        cos_t = [cos_sb[:, j * QB:(j + 1) * QB] for j in range(NB)]
        sin_t = [sin_sb[:, j * QB:(j + 1) * QB] for j in range(NB)]

        # x resident for the whole kernel, one tile per 4-chunk group
        # (512KB DMAs with 4KB contiguous lines)
        xg_t = {(j, g): xpool.tile([128, 4 * QB], BF16, tag=f"x{j}_{g}",
                                   name=f"x{j}_{g}")
                for j in range(NB) for g in range(4)}

        def xts(j):
            return [xg_t[j, d // 4][:, (d % 4) * QB:(d % 4 + 1) * QB]
                    for d in range(NDC)]
        x0_t = xts(0)

        # ---- HAM warmup: >3.4us of dummy matmuls flips the PE clock
        # gate to 8/8 before the first real matmuls arrive ----
        warm_w = wpool.tile([128, 128], BF16, tag="warmw", name="warmw")
        warm_x = wpool.tile([128, 64], BF16, tag="warmx", name="warmx")
        nc.vector.memset(warm_w[:], 0.0)
        nc.vector.memset(warm_x[:], 0.0)
        warm_ps = ps_sc.tile([128, 64], F32, tag="sc", name="warm_ps")
        for _ in range(160):
            nc.tensor.matmul(warm_ps[:], warm_w[:], warm_x[:],
                             start=True, stop=True)

        # ---- startup DMAs: the ramp is aggregate-HBM-bound (~310GB/s
        # across all queues), so transfers are issued in strict
        # need-order round-robin over the 3 DMA-capable queues, with
        # x2/x3 (needed at ~85/130us) strictly after all critical
        # bytes.  Q-phase wq/x0 groups are split into partition halves
        # across two queues so each group lands ~2x sooner. ----
        import itertools
        # phase 1 (wq + x block 0, 16 partition-half transfers) uses all
        # 3 DMA-capable queues; everything later uses ONLY sync/gpsimd:
        # dma_start instructions wait on queue-credit semaphores in the
        # issuing ENGINE's FIFO, and the scalar/ACT engine must be free
        # for rope/exp compute from ~22us.
        qcyc1 = itertools.cycle([nc.scalar, nc.sync, nc.gpsimd])
        qcyc2 = itertools.cycle([nc.sync, nc.gpsimd])

        def rr_dma(dst, src):
            next(qcyc2).dma_start(dst, src)

        def xg_src(j, g):
            c0 = (j * NDC + 4 * g) * QB
            return xt_d.ap()[:, c0:c0 + 4 * QB]

        for g in range(4):
            for dst, s in ((wq_g[g][:], wq_d.ap()[:, g * 2048:(g + 1) * 2048]),
                           (xg_t[0, g][:], xg_src(0, g))):
                next(qcyc1).dma_start(dst[0:64, :], s[0:64, :])
                next(qcyc1).dma_start(dst[64:128, :], s[64:128, :])
        ident = wpool.tile([KC, KC], BF16, tag="ident", name="ident")
        maskA = wpool.tile([KC, KC], BF16, tag="maskA", name="maskA")
        ones128 = wpool.tile([128, 1], F16, tag="ones128", name="ones128")
        ones1 = wpool.tile([1, 128], F16, tag="ones1", name="ones1")
        nc.scalar.dma_start(ident[:], idn_d.ap()[:])
        nc.scalar.dma_start(maskA[:], msk_d.ap()[:])
        nc.scalar.dma_start(ones128[:], on128_d.ap()[:])
        nc.scalar.dma_start(ones1[:], on1_d.ap()[:])
        rr_dma(cos_sb[:], cos_d.ap()[:])
        rr_dma(sin_sb[:], sin_d.ap()[:])
        rr_dma(wk_sb[:, 0:2048], wk_d.ap()[:, 0:2048])
        rr_dma(wk_sb[:, 2048:4096], wk_d.ap()[:, 2048:4096])
        rr_dma(wv_sb[:, 0:2048], wv_d.ap()[:, 0:2048])
        rr_dma(wv_sb[:, 2048:4096], wv_d.ap()[:, 2048:4096])
        for g in range(4):
            rr_dma(xg_t[1, g][:], xg_src(1, g))
        # wo resident load: [512, 2048] -> [128, (h n c)], 4KB lines
        for h in range(NH):
            rr_dma(wo_sb[:, h * 2048:(h + 1) * 2048],
                   wo_d.ap()[h * 128:(h + 1) * 128, :])
        for g in range(4):
            rr_dma(xg_t[2, g][:], xg_src(2, g))
        for g in range(4):
            rr_dma(xg_t[3, g][:], xg_src(3, g))

        # ---- persistent K^T / V for the whole sequence ----
        kT = [kvpool.tile([128, SEQ], BF16, tag=f"kT{g}", name=f"kT{g}")
              for g in range(NKV)]
        # v_sb columns: [kchunk c][kv head g] -> [:, c*256 + g*128 :+128]
        v_sb = kvpool.tile([128, (SEQ // KC) * NKV * HD], F16, tag="v", name="v_sb")
        assert v_sb.shape[1] == 4096

        def rope(dst, src_ps, cos_t, sin_t):
            """dst = src*cos2 + swap_halves(src)*sins  (dst bf16 SBUF).

            The half-swap muls must read PSUM (DVE only allows a
            partition-base shift when one operand is PSUM); the ACT
            copy in parallel frees the PSUM bank, and the remaining
            cos-mul + add run all-SBUF bf16 in 2x packed mode."""
            nc.vector.tensor_mul(dst[0:64, :], src_ps[64:128, :],
                                 sin_t[0:64, :])
            nc.vector.tensor_mul(dst[64:128, :], src_ps[0:64, :],
                                 sin_t[64:128, :])
            t0 = tpool.tile([128, QB], BF16, tag="ropesrc", name="ropesrc")
            nc.scalar.copy(t0[:], src_ps[:])
            tmp = tpool.tile([128, QB], BF16, tag="ropetmp", name="ropetmp")
            nc.vector.tensor_mul(tmp[:], t0[:], cos_t[:])
            nc.vector.tensor_add(dst[:], dst[:], tmp[:])

        def q_group(j, h, xts_, cos_t, sin_t):
            q_ps = ps_acc.tile([128, QB], F32, tag="acc", name=f"q_ps{j}_{h}")
            for d in range(NDC):
                nc.tensor.matmul(
                    q_ps[:],
                    wq_t[d][:, h * 128:(h + 1) * 128],
                    xts_[d][:], start=(d == 0), stop=(d == NDC - 1))
            qt = qpool.tile([128, QB], BF16, tag="qT", name=f"qt{j}_{h}")
            rope(qt, q_ps, cos_t, sin_t)
            return qt

        def k_group(j, g, xts_, cos_t, sin_t):
            c0 = j * QB
            k_ps = ps_acc.tile([128, QB], F32, tag="acc", name=f"k_ps{j}_{g}")
            for d in range(NDC):
                nc.tensor.matmul(
                    k_ps[:],
                    wk_t[d][:, g * 128:(g + 1) * 128],
                    xts_[d][:], start=(d == 0), stop=(d == NDC - 1))
            rope(kT[g][:, c0:c0 + QB], k_ps, cos_t, sin_t)

        def v_group(j, m, xts_):
            v_ps = ps_acc.tile([128, NKV * HD], F32, tag="acc",
                               name=f"v_ps{j}_{m}")
            for d in range(NDC):
                nc.tensor.matmul(
                    v_ps[:],
                    xts_[d][:, m * 128:(m + 1) * 128],
                    wv_t[d][:],
                    start=(d == 0), stop=(d == NDC - 1))
            kc = 4 * j + m
            with nc.allow_low_precision(reason="V in fp16 (11-bit) is plenty"):
                nc.scalar.copy(v_sb[:, kc * 256:(kc + 1) * 256], v_ps[:])

        def wo_ap(n, h):
            return wo_sb[:, h * 2048 + n * 512: h * 2048 + (n + 1) * 512]

        def attn_stream(j, nkc, qT, h):
            """Builds the per-chunk score/exp/zacc and PV issuers for
            one head, so single heads can pipeline PV two chunks behind
            the scores and head-pairs can interleave chunk-by-chunk
            (the other head's matmuls hide this head's ACT exp)."""
            g = h // 2
            o_ps = ps_att.tile([128, QB], F32, tag="att", name=f"o_ps{j}_{h}")
            zacc = zpool.tile([128, QB], F16, tag="zacc",
                              name=f"zacc{j}_{h}")
            pts = [None] * nkc
            offs = [max(0, (kc - 4 * j) * 128) for kc in range(nkc)]

            def issue_score(kc):
                off = offs[kc]
                sc_ps = ps_sc.tile([128, QB], F32, tag="sc",
                                   name=f"sc{j}_{h}_{kc}")
                if kc >= 4 * j:
                    # additive causal mask: -1e9 above the diagonal, via
                    # a tiny identity-stationary matmul into the bank
                    nc.tensor.matmul(sc_ps[:, off:off + KC], ident[:],
                                     maskA[:], start=True, stop=False)
                    nc.tensor.matmul(sc_ps[:, off:QB],
                                     kT[g][:, kc * 128:(kc + 1) * 128],
                                     qT[h][:, off:QB], start=False, stop=True)
                else:
                    nc.tensor.matmul(sc_ps[:, off:QB],
                                     kT[g][:, kc * 128:(kc + 1) * 128],
                                     qT[h][:, off:QB], start=True, stop=True)
                pt = ppool.tile([128, QB], F16, tag="pT",
                                name=f"pt{j}_{h}_{kc}")
                nc.scalar.activation(pt[:, off:QB], sc_ps[:, off:QB],
                                     mybir.ActivationFunctionType.Exp,
                                     scale=float(SCALE))
                with nc.allow_low_precision(
                        reason="softmax z accum in fp16 (11-bit) is plenty"):
                    if kc == 0:
                        nc.vector.tensor_copy(zacc[:], pt[:])
                    else:
                        nc.vector.tensor_add(zacc[:, off:QB],
                                             zacc[:, off:QB],
                                             pt[:, off:QB])
                pts[kc] = pt

            def issue_pv(kc):
                off = offs[kc]
                nc.tensor.matmul(o_ps[:, off:QB],
                                 v_sb[:, kc * 256 + g * 128:
                                      kc * 256 + (g + 1) * 128],
                                 pts[kc][:, off:QB], start=(kc == 0),
                                 stop=(kc == nkc - 1))

            def finish():
                # stage unnormalized O' (frees the PSUM bank quickly);
                # the z finalize is issued separately (z_fin)
                o_sb = ospool.tile([128, QB], BF16, tag="osb",
                                   name=f"o_sb{j}_{h}")
                nc.scalar.copy(o_sb[:], o_ps[:])
                return (o_sb, zacc)

            return issue_score, issue_pv, finish

        def attn_head(j, nkc, qT, h):
            sc, pv, fin = attn_stream(j, nkc, qT, h)
            sc(0)
            sc(1)
            for kc in range(2, nkc):
                sc(kc)
                pv(kc - 2)
            pv(nkc - 2)
            pv(nkc - 1)
            return fin()

        def attn_head_pair(j, nkc, qT, hA, hB):
            """Two heads interleaved chunk-by-chunk: each head's exp
            hides behind the other's matmuls (used for the last block,
            which has no other dense work to interleave with)."""
            scA, pvA, finA = attn_stream(j, nkc, qT, hA)
            scB, pvB, finB = attn_stream(j, nkc, qT, hB)
            scA(0)
            scB(0)
            for kc in range(1, nkc):
                scA(kc)
                scB(kc)
                pvA(kc - 1)
                pvB(kc - 1)
            pvA(nkc - 1)
            pvB(nkc - 1)
            return finA(), finB()

        def z_fin(j, h, zacc):
            # z row = ones^T @ zacc (partition reduce), then reshape the z
            # row to [128,4] so the reciprocal runs on all 128 DVE lanes;
            # reshape hops ride the GPSIMD queue, which is idle once the
            # x loads finish (sync carries output writes and would add
            # multi-us queueing latency at the tail)
            z_ps = ps_sc.tile([1, QB], F32, tag="sc", name=f"z_ps{j}_{h}")
            nc.tensor.matmul(z_ps[:], ones128[:], zacc[:],
                             start=True, stop=True)
            z_sb = npool.tile([1, QB], F32, tag="z", bufs=4,
                              name=f"z_sb{j}_{h}")
            nc.scalar.copy(z_sb[:], z_ps[:])
            zc = npool.tile([128, QB // 128], F32, tag="zc", bufs=4,
                            name=f"zc{j}_{h}")
            nc.gpsimd.dma_start(zc[:], z_sb[:])
            rzc = npool.tile([128, QB // 128], F16, tag="rzc", bufs=4,
                             name=f"rzc{j}_{h}")
            with nc.allow_low_precision(
                    reason="1/z in fp16 (11-bit mantissa) is plenty"):
                nc.vector.reciprocal(rzc[:], zc[:])
            rz = npool.tile([1, QB], F16, tag="rz", bufs=6,
                            name=f"rz{j}_{h}")
            nc.gpsimd.dma_start(rz[:], rzc[:])
            return rz

        def norm_head(j, h, o_sb, rz):
            # o_sb *= broadcast(1/z) (in place)
            zb_ps = ps_sc.tile([128, QB], F32, tag="sc", name=f"zb{j}_{h}")
            nc.tensor.matmul(zb_ps[:], ones1[:], rz[:], start=True, stop=True)
            nc.vector.tensor_mul(o_sb[:], o_sb[:], zb_ps[:])

        def op_group(j, n, mp, oT, pool, tag, heads, start, stop, evac):
            """Issue outproj matmuls for heads `heads` of column group
            (n, mp) into 2 PSUM banks from `pool`; returns the banks."""
            op_ps = [pool.tile([128, 512], F32, tag=tag,
                               name=f"op{j}_{n}_{mp}_{m}")
                     for m in range(2)]
            return op_cont(j, n, mp, oT, op_ps, heads, start, stop, evac)

        def op_cont(j, n, mp, oT, op_ps, heads, start, stop, evac):
            c0 = j * QB
            for h in heads:
                for mi in range(2):
                    m = 2 * mp + mi
                    nc.tensor.matmul(
                        op_ps[mi][:],
                        oT[h][:, m * 128:(m + 1) * 128],
                        wo_ap(n, h),
                        start=(h == heads[0] and start),
                        stop=(h == heads[-1] and stop))
            if evac:
                for mi in range(2):
                    m = 2 * mp + mi
                    ob = obpool.tile([128, 512], BF16, tag="ob",
                                     name=f"ob{j}_{n}_{m}")
                    # split PSUM->SBUF evacuations between ACT and DVE
                    if mi == 0:
                        nc.scalar.copy(ob[:], op_ps[mi][:])
                    else:
                        nc.vector.tensor_copy(ob[:], op_ps[mi][:])
                    oeng = nc.sync if (n + mi) % 2 == 0 else nc.gpsimd
                    oeng.dma_start(
                        out_d.ap()[c0 + m * 128: c0 + (m + 1) * 128,
                                   n * 512:(n + 1) * 512], ob[:])
            return op_ps

        def outproj_block(j, oT, rzs, skip_norm=(), ns=(0, 1, 2, 3)):
            for h in range(NH):
                if h not in skip_norm:
                    norm_head(j, h, oT[h], rzs[h])
            for n in ns:
                for mp in range(2):
                    op_group(j, n, mp, oT, ps_acc, "acc", list(range(NH)),
                             True, True, True)

        # ---- software pipeline ----
        # Block 0 QKV: all 4 q-head accumulations interleaved per
        # d-chunk so the PE consumes each x/wq chunk-pair as it lands.
        cos0 = cos_t[0]
        sin0 = sin_t[0]
        q_ps0 = [ps_acc.tile([128, QB], F32, tag="acc", name=f"q_ps0_{h}")
                 for h in range(3)]
        q_ps0.append(ps_att.tile([128, QB], F32, tag="att", name="q_ps0_3"))
        for d in range(NDC):
            for h in range(NH):
                nc.tensor.matmul(
                    q_ps0[h][:],
                    wq_t[d][:, h * 128:(h + 1) * 128],
                    x0_t[d][:], start=(d == 0), stop=(d == NDC - 1))
        qT_cur = []
        for h in range(NH):
            qt = qpool.tile([128, QB], BF16, tag="qT", name=f"qt0_{h}")
            rope(qt, q_ps0[h], cos0, sin0)
            qT_cur.append(qt)
        for g in range(NKV):
            k_group(0, g, x0_t, cos0, sin0)
        for m in range(4):
            v_group(0, m, x0_t)

        LAST = NB - 1
        oT3, rz3 = [], []
        for j in range(NB - 1):
            nkc = 4 * (j + 1)
            xtsn = xts(j + 1)
            cosn = cos_t[j + 1]
            sinn = sin_t[j + 1]
            oT_cur = [attn_head(j, nkc, qT_cur, 0),
                      attn_head(j, nkc, qT_cur, 1)]
            rz_cur = [z_fin(j, 0, oT_cur[0][1])]
            qT_next = [q_group(j + 1, 0, xtsn, cosn, sinn)]
            oT_cur.append(attn_head(j, nkc, qT_cur, 2))
            rz_cur.append(z_fin(j, 1, oT_cur[1][1]))
            qT_next.append(q_group(j + 1, 1, xtsn, cosn, sinn))
            oT_cur.append(attn_head(j, nkc, qT_cur, 3))
            rz_cur.append(z_fin(j, 2, oT_cur[2][1]))
            qT_next.append(q_group(j + 1, 2, xtsn, cosn, sinn))
            qT_next.append(q_group(j + 1, 3, xtsn, cosn, sinn))
            rz_cur.append(z_fin(j, 3, oT_cur[3][1]))
            for g in range(NKV):
                k_group(j + 1, g, xtsn, cosn, sinn)
            for m in range(4):
                v_group(j + 1, m, xtsn)
            qT_cur = qT_next
            oT_sb = [o for o, _ in oT_cur]
            if j == NB - 2:
                # the last block's heads run as self-covering pairs; all
                # z chains overlap outproj(j) halves, so outproj(LAST)
                # starts unblocked
                a0, a1 = attn_head_pair(LAST, 4 * NB, qT_cur, 0, 1)
                oT3 += [a0, a1]
                rz3.append(z_fin(LAST, 0, oT3[0][1]))
                rz3.append(z_fin(LAST, 1, oT3[1][1]))
                outproj_block(j, oT_sb, rz_cur, ns=(0, 1))
                a2, a3 = attn_head_pair(LAST, 4 * NB, qT_cur, 2, 3)
                oT3 += [a2, a3]
                rz3.append(z_fin(LAST, 2, oT3[2][1]))
                rz3.append(z_fin(LAST, 3, oT3[3][1]))
                outproj_block(j, oT_sb, rz_cur,
                              skip_norm=(0, 1, 2, 3), ns=(2, 3))
            else:
                outproj_block(j, oT_sb, rz_cur)
        outproj_block(LAST, [o for o, _ in oT3], rz3)

    nc.compile()
    return nc


_NC_CACHE = None


def _get_nc():
    global _NC_CACHE
    if _NC_CACHE is None:
        _NC_CACHE = _build_nc()
    return _NC_CACHE


def _host_prep(inputs):
    """Build the 8 per-core input maps from the full problem inputs."""
    hs = np.asarray(inputs["hidden_state"], dtype=np.float32)
    cos = np.asarray(inputs["freq_cos"], dtype=np.float32)[0, :, 0, :]  # [S,64]
    sin = np.asarray(inputs["freq_sin"], dtype=np.float32)[0, :, 0, :]
    wq = np.asarray(inputs["wq"], dtype=np.float32)
    wk = np.asarray(inputs["wk"], dtype=np.float32)
    wv = np.asarray(inputs["wv"], dtype=np.float32)
    wo = np.asarray(inputs["wo"], dtype=np.float32)

    perm = np.concatenate([np.arange(0, HD, 2), np.arange(1, HD, 2)])  # [128]

    cos2 = np.empty((HD, SEQ), dtype=np.float32)
    sins = np.empty((HD, SEQ), dtype=np.float32)
    cos2[:HALF] = cos.T
    cos2[HALF:] = cos.T
    sins[:HALF] = -sin.T
    sins[HALF:] = sin.T
    cos2 = cos2.astype(ml_dtypes.bfloat16)
    sins = sins.astype(ml_dtypes.bfloat16)

    ki = np.arange(KC)
    # additive causal mask for diagonal chunks: key k > query c -> -1e9
    maskadd = np.where(ki[:, None] > ki[None, :], -1e9,
                       0.0).astype(ml_dtypes.bfloat16)
    ident = np.eye(KC, dtype=ml_dtypes.bfloat16)
    ones128 = np.ones((128, 1), dtype=np.float16)
    ones1 = np.ones((1, 128), dtype=np.float16)

    def tile_pdc(w):
        # [2048, C] -> [128, 16*C]: row p holds chunks d=0..15 contiguously
        c = w.shape[1]
        return np.ascontiguousarray(
            w.reshape(NDC, 128, c).transpose(1, 0, 2).reshape(128, NDC * c))

    # x^T -> [p][j][d][c] so block-j 4-chunk groups are 4KB-contiguous
    xTs = []
    for b in range(BS):
        xT = hs[b].T.astype(ml_dtypes.bfloat16)          # [dim, seq]
        x4 = xT.reshape(NDC, 128, NB, QB).transpose(1, 2, 0, 3)
        xTs.append(np.ascontiguousarray(x4.reshape(128, NB * NDC * QB)))

    in_maps = []
    for c in range(NCORES):
        b, r = divmod(c, TP)
        qcols = np.concatenate(
            [(4 * r + h) * HD + perm for h in range(NH)])
        kcols = np.concatenate(
            [(NKV * r + g) * HD + perm for g in range(NKV)])
        vcols = np.concatenate(
            [(NKV * r + g) * HD + np.arange(HD) for g in range(NKV)])
        worows = np.concatenate(
            [(4 * r + h) * HD + np.arange(HD) for h in range(NH)])
        in_maps.append({
            "xt": xTs[b],
            "wq": tile_pdc(wq[:, qcols].astype(ml_dtypes.bfloat16)),
            "wk": tile_pdc(wk[:, kcols].astype(ml_dtypes.bfloat16)),
            "wv": tile_pdc(wv[:, vcols].astype(ml_dtypes.bfloat16)),
            "wo": np.ascontiguousarray(wo[worows, :]).astype(ml_dtypes.bfloat16),
            "cos2": cos2,
            "sins": sins,
            "maskadd": maskadd,
            "ident": ident,
            "ones128": ones128,
            "ones1": ones1,
        })
    return in_maps


def _run(inputs, trace=False, **trace_kwargs):
    nc = _get_nc()
    in_maps = _host_prep(inputs)
    res = run_bass_kernel_spmd(nc, in_maps, list(range(NCORES)),
                               trace=trace, **trace_kwargs)
    out = np.zeros((BS, SEQ, DIM), dtype=np.float32)
    for c in range(NCORES):
        out[c // TP] += np.asarray(res.results[c]["out"], dtype=np.float32)
    return out, res


def kernel(**inputs) -> np.ndarray:
    out, _ = _run(inputs, trace=False)
    return out
